# revision 28
# baseline (speedup 1.0000x reference)
"""Trainium2 Bass kernel for a dense transformer layer (attention + FFN + 2 LayerNorms).

Problem shapes: x [4, 2048, 1024], d_model=1024, heads=16 (hd=64), d_ff=4096.

Sharding (8 cores): core c handles batch b = c//2.  The core PAIR (2b, 2b+1)
splits the layer two ways:
  - attention is HEAD-sharded: core half=c%2 computes Q/K/V and attention for
    its 8 heads over ALL 2048 tokens of the batch (no duplicated K/V work);
  - everything after attention (Wo, LN1, FFN, LN2) is TOKEN-sharded: each
    core owns 1024 tokens (host permutes x so own tokens are columns 0:1023).
A tiny pairwise AllGather per 2-head group moves the normalized attention
context for the peer's token half across the pair (4 x 256KB, overlapped with
the remaining attention groups).  The peer's contribution enters Wo as extra
stationary row-blocks whose unused slot rows are zeroed host-side, so no
per-core control flow is needed (pure SPMD).

Layout: activations feature-major (features on partitions, tokens free).
Scores are computed per head with contraction 64 (S^T = K_h Q_h^T, keys on
partitions), exp on the scalar engine over [128,1024] PSUM tiles, ctx via
V_aug (ones column -> softmax denominator for free).  QKV projection of group
g+1 overlaps the scalar-bound attention of group g on the PE.
"""

import os
import numpy as np

import concourse.bass as bass
import concourse.tile as tile
from concourse import bacc, mybir
from concourse import bass_utils

BF16 = mybir.dt.bfloat16
F32 = mybir.dt.float32
AF = mybir.ActivationFunctionType
OP = mybir.AluOpType

D = 1024          # d_model
S = 2048          # full sequence per batch
T = 1024          # tokens owned per core (post-attention)
H = 16            # total heads
HG = 4            # local 2-head groups per core (8 heads)
HD = 64           # head dim
F = 4096          # ffn hidden
P = 128
DT = D // P       # 8 feature tiles
KT = S // P       # 16 key-token tiles
FT = F // P       # 32 hidden tiles
N_CORES = 8
EPS = 1e-5
GROUPS = [[0, 1], [2, 3], [4, 5], [6, 7]]

_CACHED = {}


def _build_program():
    nc = bacc.Bacc("TRN2", target_bir_lowering=False, debug=False,
                   num_devices=N_CORES)

    tens = {}

    def di(name, shape, dtype=BF16):
        tens[name] = nc.dram_tensor(name, shape, dtype, kind="ExternalInput")

    di("xT", [D, S])
    di("wq", [D, HG, P]); di("wk", [D, HG, P]); di("wv", [D, HG, P])
    di("wo", [12 * P, D])
    di("w1", [D, F]); di("w2", [F, D])
    for nm in ["bq_p", "bk_p", "bv_p"]:
        di(nm, [P, HG], F32)
    for nm in ["bo_p", "b2_p", "g1_p", "be1_p"]:
        di(nm, [P, DT], F32)
    di("b1_p", [P, FT], F32)
    di("g2_d", [D], F32); di("be2_d", [D], F32)
    di("ident_d", [P, P]); di("ones_row_d", [1, P]); di("ones_col_d", [P, 1])
    di("selD_d", [HD + 1, HD])
    tens["out"] = nc.dram_tensor("out", [T, D], F32, kind="ExternalOutput")

    with tile.TileContext(nc) as tc:
        _trace_kernel(nc, tc, tens)
    nc.compile()
    return nc


def _trace_kernel(nc, tc, t):
    xT, wq, wk, wv, wo, w1, w2 = (t["xT"], t["wq"], t["wk"], t["wv"], t["wo"],
                                  t["w1"], t["w2"])
    out = t["out"]

    from contextlib import ExitStack
    es = ExitStack()
    with es:
        dram = es.enter_context(tc.tile_pool(name="dram", bufs=1, space="DRAM"))

        const = es.enter_context(tc.tile_pool(name="const", bufs=1))
        ident = const.tile([P, P], BF16, tag="ident", name="ident")
        nc.sync.dma_start(out=ident, in_=t["ident_d"][:, :])
        ones_row = const.tile([1, P], BF16, tag="onesr", name="onesr")
        nc.sync.dma_start(out=ones_row, in_=t["ones_row_d"][:, :])
        ones_col = const.tile([P, 1], BF16, tag="onesc", name="onesc")
        nc.sync.dma_start(out=ones_col, in_=t["ones_col_d"][:, :])
        selD = const.tile([HD + 1, HD], BF16, tag="selD", name="selD")
        nc.sync.dma_start(out=selD, in_=t["selD_d"][:, :])
        biases = {}
        for name in ["bq_p", "bk_p", "bv_p"]:
            bt = const.tile([P, HG], F32, tag=name)
            nc.sync.dma_start(out=bt, in_=t[name][:, :])
            biases[name] = bt
        for name in ["bo_p", "b2_p", "g1_p", "be1_p"]:
            bt = const.tile([P, DT], F32, tag=name)
            nc.sync.dma_start(out=bt, in_=t[name][:, :])
            biases[name] = bt
        eps_sb = const.tile([P, 1], F32, tag="eps", name="eps")
        nc.vector.memset(eps_sb[:], EPS)
        b1_sb = const.tile([P, FT], F32, tag="b1", name="b1")
        nc.sync.dma_start(out=b1_sb, in_=t["b1_p"][:, :])

        # long-lived pools (outlive phase W into the FFN) must open before
        # the phase-F/W pools below them on the allocation stack
        hT_pool = es.enter_context(tc.tile_pool(name="hT", bufs=1))
        hT = [hT_pool.tile([P, T], BF16, tag=f"hT{i}", name=f"hT{i}")
              for i in range(DT)]
        wx_pool = es.enter_context(tc.tile_pool(name="wx", bufs=12))
        w1p_pool = es.enter_context(tc.tile_pool(name="w1p", bufs=8))

        # pools that die after phase W (freed before the FFN needs SBUF)
        es_fw = ExitStack()
        # x resident, feature-major, own tokens first (cols 0:1024)
        xsb_pool = es_fw.enter_context(tc.tile_pool(name="xsb", bufs=1))
        xsb = []
        for dt_ in range(DT):
            xt_ = xsb_pool.tile([P, S], BF16, tag=f"x{dt_}", name=f"x{dt_}")
            nc.sync.dma_start(out=xt_, in_=xT[dt_ * P:(dt_ + 1) * P, :])
            xsb.append(xt_)

        # QKV weight slices for own 8 heads: [D, HG, 128] -> per-din tiles
        wqkv_pool = es_fw.enter_context(tc.tile_pool(name="wqkv", bufs=1))
        wsl = {}
        for nm, wd in (("q", wq), ("k", wk), ("v", wv)):
            tiles = []
            for din in range(DT):
                wt = wqkv_pool.tile([P, HG, P], BF16, tag=f"w{nm}{din}",
                                    name=f"w{nm}{din}")
                nc.sync.dma_start(out=wt, in_=wd[din * P:(din + 1) * P, :, :])
                tiles.append(wt)
            wsl[nm] = tiles

        # normalized ctx per group (own token half only), persists until Wo
        ctxn_pool = es_fw.enter_context(tc.tile_pool(name="ctxn", bufs=1))
        ctxn = [ctxn_pool.tile([P, T], BF16, tag=f"cn{g}", name=f"cn{g}")
                for g in range(HG)]

        # AllGather bounce buffers (HBM)
        agin = [dram.tile([P, T], BF16, tag=f"agi{g}", name=f"agi{g}")
                for g in range(HG)]
        agout = [dram.tile([2, P, T], BF16, tag=f"ago{g}", name=f"ago{g}")
                 for g in range(HG)]

        # =============== Phase F: fused QKV + attention per group ========
        with tc.tile_pool(name="kbuf", bufs=2) as kbuf_pool, \
             tc.tile_pool(name="qbuf", bufs=2) as qbuf_pool, \
             tc.tile_pool(name="vbuf", bufs=2) as vbuf_pool, \
             tc.tile_pool(name="vtmp", bufs=2) as vtmp_pool, \
             tc.tile_pool(name="ctx", bufs=2) as ctx_pool, \
             tc.tile_pool(name="pbuf", bufs=3) as pbuf_pool, \
             tc.tile_pool(name="rcp", bufs=2) as rcp_pool, \
             tc.tile_pool(name="psS", bufs=2, space="PSUM") as psS, \
             tc.tile_pool(name="psC", bufs=2, space="PSUM") as psC, \
             tc.tile_pool(name="psA", bufs=1, space="PSUM") as psA, \
             tc.tile_pool(name="psX", bufs=1, space="PSUM") as psX:

            for g in range(HG):
                # ---- projections for group g (2 heads = 128 features) ----
                ksb = kbuf_pool.tile([P, S], BF16, tag="kb", name="kb")
                # Q staged zero-padded per head: sibling head rows zero so
                # the full [128,128] K stationary tiles cancel them
                qz = [qbuf_pool.tile([P, S], BF16, tag=f"qz{hh}",
                                     name=f"qz{hh}") for hh in range(2)]
                for hh in range(2):
                    nc.vector.memset(qz[hh][:], 0.0)
                vsb = vbuf_pool.tile([P, KT, 2, HD + 1], BF16, tag="vb",
                                     name="vb")

                def proj(wtiles, bias, dest_cb, g=g):
                    for ch in range(S // 512):
                        ps = psA.tile([P, 512], F32, tag="psA", name="psA")
                        for din in range(DT):
                            nc.tensor.matmul(
                                ps[:], wtiles[din][:, g, :],
                                xsb[din][:, ch * 512:(ch + 1) * 512],
                                start=(din == 0), stop=(din == DT - 1))
                        dest_cb(ps, ch)

                def k_evict(ps, ch, ksb=ksb, g=g):
                    nc.vector.tensor_scalar(
                        out=ksb[:, ch * 512:(ch + 1) * 512], in0=ps[:],
                        scalar1=biases["bk_p"][:, g:g + 1], scalar2=None,
                        op0=OP.add)

                def q_evict(ps, ch, qz=qz, g=g):
                    for hh in range(2):
                        r0 = hh * HD
                        nc.vector.tensor_scalar(
                            out=qz[hh][r0:r0 + HD, ch * 512:(ch + 1) * 512],
                            in0=ps[r0:r0 + HD, :],
                            scalar1=biases["bq_p"][r0:r0 + HD, g:g + 1],
                            scalar2=None, op0=OP.add)

                def v_evict(ps, ch, vsb=vsb, g=g):
                    vt = vtmp_pool.tile([P, 512], BF16, tag="vt", name="vt")
                    nc.vector.tensor_scalar(
                        out=vt[:], in0=ps[:],
                        scalar1=biases["bv_p"][:, g:g + 1], scalar2=None,
                        op0=OP.add)
                    for hh in range(2):
                        idsl = ident[hh * HD:(hh + 1) * HD,
                                     hh * HD:(hh + 1) * HD]
                        for st in range(4):
                            pt = psX.tile([P, HD], BF16, tag="psX",
                                          name="psX")
                            nc.tensor.transpose(
                                pt[:],
                                vt[hh * HD:(hh + 1) * HD,
                                   st * P:(st + 1) * P], idsl)
                            nc.vector.tensor_copy(
                                vsb[:, ch * 4 + st, hh, 0:HD], pt[:])

                proj(wsl["k"], "bk_p", k_evict)
                proj(wsl["v"], "bv_p", v_evict)
                nc.vector.memset(vsb[:, :, :, HD:HD + 1], 1.0)
                proj(wsl["q"], "bq_p", q_evict)

                # ---- attention for the 2 heads over all 2048 queries ----
                # pass 0: own token half -> ctxn[g]; pass 1: peer -> send
                send = ctx_pool.tile([P, T], BF16, tag="send", name="send")
                for hh in range(2):
                    r0 = hh * HD
                    for ps_ in range(2):
                        p0 = ps_ * 1024
                        cpss = [psC.tile([HD + 1, 512], F32, tag="cps",
                                         name="cps") for _ in range(2)]
                        for jp in range(KT // 2):
                            j0, j1 = 2 * jp, 2 * jp + 1
                            for qc in range(2):
                                c0 = p0 + qc * 512
                                sps = psS.tile([P, 2, 512], F32, tag="sps",
                                               name="sps")
                                nc.tensor.matmul(
                                    sps[:, 0, :],
                                    ksb[:, j0 * P:(j0 + 1) * P],
                                    qz[hh][:, c0:c0 + 512],
                                    start=True, stop=True)
                                nc.tensor.matmul(
                                    sps[:, 1, :],
                                    ksb[:, j1 * P:(j1 + 1) * P],
                                    qz[hh][:, c0:c0 + 512],
                                    start=True, stop=True)
                                pT = pbuf_pool.tile([P, 2, 512], BF16,
                                                    tag="pT", name="pT")
                                nc.scalar.activation(pT[:], sps[:], AF.Exp)
                                nc.tensor.matmul(
                                    cpss[qc][:], vsb[:, j0, hh, :],
                                    pT[:, 0, :],
                                    start=(jp == 0), stop=False)
                                nc.tensor.matmul(
                                    cpss[qc][:], vsb[:, j1, hh, :],
                                    pT[:, 1, :],
                                    start=False, stop=(jp == KT // 2 - 1))
                        # evict + normalize by softmax denominator (row 64)
                        dest = ctxn[g] if ps_ == 0 else send
                        for qc in range(2):
                            ctx_sb = ctx_pool.tile([HD + 1, 512], BF16,
                                                   tag="cs", name="cs")
                            nc.vector.tensor_copy(ctx_sb[:], cpss[qc][:])
                            dn = psA.tile([P, 512], F32, tag="psA",
                                          name="psA")
                            nc.tensor.matmul(
                                dn[0:HD, :], selD[:, :], ctx_sb[:],
                                start=True, stop=True)
                            rc = rcp_pool.tile([HD, 512], F32, tag="rc",
                                               name="rc")
                            nc.vector.reciprocal(rc[:], dn[0:HD, :])
                            nc.vector.tensor_tensor(
                                out=dest[r0:r0 + HD,
                                         qc * 512:(qc + 1) * 512],
                                in0=ctx_sb[0:HD, :],
                                in1=rc[:], op=OP.mult)

                # ---- ship peer's token half to the pair core ------------
                nc.gpsimd.dma_start(agin[g][:, :], send[:])
                nc.gpsimd.collective_compute(
                    "AllGather", OP.bypass, replica_groups=GROUPS,
                    ins=[agin[g][:, :].opt()],
                    outs=[agout[g][:, :, :].opt()])

            # Fence: a 5th tiny AllGather.  The tile framework makes each
            # collective's input writer wait until all PRIOR collectives'
            # data has ARRIVED (the trigger instruction itself completes
            # early), so the fence writer + gpsimd program order make the
            # agout readbacks below race-free.
            fence_sb = const.tile([P, 8], BF16, tag="fsb", name="fsb")
            nc.vector.memset(fence_sb[:], 0.0)
            fence_in = dram.tile([P, 8], BF16, tag="fin", name="fin")
            fence_out = dram.tile([2, P, 8], BF16, tag="fout", name="fout")
            nc.gpsimd.dma_start(fence_in[:, :], fence_sb[:])
            nc.gpsimd.collective_compute(
                "AllGather", OP.bypass, replica_groups=GROUPS,
                ins=[fence_in[:, :].opt()],
                outs=[fence_out[:, :, :].opt()])

        # =============== Phase W: Wo + residual + LN1 ===================
        with tc.tile_pool(name="asb", bufs=1) as asb_pool, \
             tc.tile_pool(name="zT", bufs=1) as zT_pool, \
             tc.tile_pool(name="ln1", bufs=2) as ln1_pool, \
             tc.tile_pool(name="psW", bufs=3, space="PSUM") as psW, \
             tc.tile_pool(name="psStat", bufs=1, space="PSUM") as psStat, \
             tc.tile_pool(name="psBc", bufs=1, space="PSUM") as psBc:
            wo_sb = [wx_pool.tile([P, D], BF16, tag="wx", name="wx")
                     for _ in range(12)]
            for i in range(12):
                nc.sync.dma_start(out=wo_sb[i], in_=wo[i * P:(i + 1) * P, :])
            # FFN hb=0 W1 preload (overlaps W phase)
            w1b0 = [w1p_pool.tile([P, D], BF16, tag="w1p", name="w1p")
                    for _ in range(DT)]
            for i in range(DT):
                nc.sync.dma_start(out=w1b0[i], in_=w1[i * P:(i + 1) * P,
                                                      0:1024])

            # peer ctx contributions from the AllGathers (both slots; the
            # useless slot's wo rows are zero)
            asb = []
            for g in range(HG):
                for s in range(2):
                    a = asb_pool.tile([P, T], BF16, tag=f"a{g}{s}",
                                      name=f"a{g}{s}")
                    # gpsimd queue: executes after the fence writer above,
                    # i.e. after every AllGather's data has arrived
                    nc.gpsimd.dma_start(a[:], agout[g][s, :, :])
                    asb.append(a)
            # moving operands for Wo in wo_sb row order:
            #   rows 0:512   -> own ctxn groups 0..3 (own tokens = cols 0:T)
            #   rows 512:1536 -> agout g0 s0, g0 s1, g1 s0, ... (g3 last so
            #   the last AllGather's wait overlaps the first 10 matmuls)
            movs = [ctxn[g][:] for g in range(HG)] + asb

            zT = [zT_pool.tile([P, T], BF16, tag=f"zT{i}", name=f"zT{i}")
                  for i in range(DT)]
            for ch in range(T // 512):
                for dout in range(DT):
                    ps = psW.tile([P, 512], F32, tag="psW", name="psW")
                    for din in range(12):
                        nc.tensor.matmul(
                            ps[:], wo_sb[din][:, dout * P:(dout + 1) * P],
                            movs[din][:, ch * 512:(ch + 1) * 512],
                            start=(din == 0), stop=(din == 11))
                    # z = attn_out + bo + x_resid
                    nc.vector.scalar_tensor_tensor(
                        zT[dout][:, ch * 512:(ch + 1) * 512], ps[:],
                        biases["bo_p"][:, dout:dout + 1],
                        xsb[dout][:, ch * 512:(ch + 1) * 512],
                        op0=OP.add, op1=OP.add)

            # ---- LN1 (feature-major; stats over partitions via PE) -----
            for ch in range(T // 512):
                sl = slice(ch * 512, (ch + 1) * 512)
                sum_ps = psStat.tile([1, 512], F32, tag="s", name="s")
                sq_ps = psStat.tile([1, 512], F32, tag="q", name="q")
                for dt_ in range(DT):
                    zsq = ln1_pool.tile([P, 512], BF16, tag="zsq",
                                        name="zsq")
                    nc.scalar.activation(zsq[:], zT[dt_][:, sl], AF.Square)
                    nc.tensor.matmul(sum_ps[:], ones_col[:], zT[dt_][:, sl],
                                     start=(dt_ == 0), stop=(dt_ == DT - 1))
                    nc.tensor.matmul(sq_ps[:], ones_col[:], zsq[:],
                                     start=(dt_ == 0), stop=(dt_ == DT - 1))
                mean = ln1_pool.tile([1, 512], F32, tag="mean", name="mean")
                nc.scalar.mul(mean[:], sum_ps[:], 1.0 / D)
                msq = ln1_pool.tile([1, 512], F32, tag="msq", name="msq")
                nc.scalar.mul(msq[:], sq_ps[:], 1.0 / D)
                m2 = ln1_pool.tile([1, 512], F32, tag="m2", name="m2")
                nc.vector.tensor_mul(m2[:], mean[:], mean[:])
                var = ln1_pool.tile([1, 512], F32, tag="var", name="var")
                nc.vector.tensor_sub(var[:], msq[:], m2[:])
                std = ln1_pool.tile([1, 512], F32, tag="std", name="std")
                nc.scalar.activation(std[:], var[:], AF.Sqrt,
                                     bias=eps_sb[0:1, :])
                rstd = ln1_pool.tile([1, 512], F32, tag="rstd", name="rstd")
                nc.vector.reciprocal(rstd[:], std[:])
                mean_r = ln1_pool.tile([1, 512], BF16, tag="meanr",
                                       name="meanr")
                nc.vector.tensor_copy(mean_r[:], mean[:])
                rstd_r = ln1_pool.tile([1, 512], BF16, tag="rstdr",
                                       name="rstdr")
                nc.vector.tensor_copy(rstd_r[:], rstd[:])
                bm_ps = psBc.tile([P, 512], F32, tag="bm", name="bm")
                nc.tensor.matmul(bm_ps[:], ones_row[:], mean_r[:],
                                 start=True, stop=True)
                br_ps = psBc.tile([P, 512], F32, tag="br", name="br")
                nc.tensor.matmul(br_ps[:], ones_row[:], rstd_r[:],
                                 start=True, stop=True)
                bm = ln1_pool.tile([P, 512], F32, tag="bm_sb", name="bm_sb")
                nc.scalar.copy(bm[:], bm_ps[:])
                br = ln1_pool.tile([P, 512], F32, tag="br_sb", name="br_sb")
                nc.scalar.copy(br[:], br_ps[:])
                for dt_ in range(DT):
                    tmp = ln1_pool.tile([P, 512], F32, tag="n1", name="n1")
                    nc.vector.scalar_tensor_tensor(
                        tmp[:], zT[dt_][:, sl],
                        1.0, bm[:], op0=OP.mult, op1=OP.subtract)
                    tmp2 = ln1_pool.tile([P, 512], F32, tag="n2", name="n2")
                    nc.vector.scalar_tensor_tensor(
                        tmp2[:], tmp[:],
                        biases["g1_p"][:, dt_:dt_ + 1], br[:],
                        op0=OP.mult, op1=OP.mult)
                    nc.scalar.activation(
                        hT[dt_][:, sl], tmp2[:], AF.Identity,
                        bias=biases["be1_p"][:, dt_:dt_ + 1])

        es_fw.close()   # free x / QKV weights / ctx SBUF before the FFN

        # =============== Phase 4: FFN + residual + fused LN2/out ========
        with tc.tile_pool(name="wxf", bufs=20) as wxf_pool, \
             tc.tile_pool(name="z2T", bufs=1) as z2T_pool, \
             tc.tile_pool(name="t1", bufs=12) as t1_pool, \
             tc.tile_pool(name="ztmp", bufs=2) as ztmp_pool, \
             tc.tile_pool(name="o2", bufs=1) as o2_pool, \
             tc.tile_pool(name="tm", bufs=2) as tm_pool, \
             tc.tile_pool(name="lnc", bufs=1) as lnc_pool, \
             tc.tile_pool(name="ln2", bufs=2) as ln2_pool, \
             tc.tile_pool(name="psF1", bufs=3, space="PSUM") as psF1, \
             tc.tile_pool(name="psF2", bufs=3, space="PSUM") as psF2, \
             tc.tile_pool(name="psT5", bufs=2, space="PSUM") as psT5:
            z2T = [z2T_pool.tile([P, T], BF16, tag=f"z2T{i}", name=f"z2T{i}")
                   for i in range(DT)]
            out2 = [o2_pool.tile([P, T], F32, tag=f"o2{i}", name=f"o2{i}")
                    for i in range(DT)]
            g2_bc = lnc_pool.tile([P, D], F32, tag="g2bc", name="g2bc")
            nc.sync.dma_start(out=g2_bc, in_=bass.AP(
                tensor=t["g2_d"], offset=0, ap=[[0, P], [1, D]]))
            be2_bc = lnc_pool.tile([P, D], F32, tag="be2bc", name="be2bc")
            nc.sync.dma_start(out=be2_bc, in_=bass.AP(
                tensor=t["be2_d"], offset=0, ap=[[0, P], [1, D]]))

            def phase5(nts):
                for nt in nts:
                    z2 = tm_pool.tile([P, D], F32, tag="z2tm", name="z2tm")
                    for dt_ in range(DT):
                        pt = psT5.tile([P, P], BF16, tag="psT5", name="psT5")
                        nc.tensor.transpose(
                            pt[:], z2T[dt_][:, nt * P:(nt + 1) * P],
                            ident[:])
                        nc.scalar.copy(z2[:, dt_ * P:(dt_ + 1) * P], pt[:])
                    stats = ln2_pool.tile([P, 2, 6], F32, tag="st", name="st")
                    for gg in range(2):
                        nc.vector.bn_stats(out=stats[:, gg, :],
                                           in_=z2[:, gg * 512:(gg + 1) * 512])
                    mv = ln2_pool.tile([P, 2], F32, tag="mv", name="mv")
                    nc.vector.bn_aggr(out=mv[:], in_=stats[:])
                    std = ln2_pool.tile([P, 1], F32, tag="std2", name="std2")
                    nc.scalar.activation(std[:], mv[:, 1:2], AF.Sqrt,
                                         bias=eps_sb[:])
                    rstd = ln2_pool.tile([P, 1], F32, tag="rstd2",
                                         name="rstd2")
                    nc.vector.reciprocal(rstd[:], std[:])
                    xn = ln2_pool.tile([P, D], F32, tag="xn", name="xn")
                    nc.vector.tensor_scalar(
                        out=xn[:], in0=z2[:], scalar1=mv[:, 0:1],
                        scalar2=rstd[:], op0=OP.subtract, op1=OP.mult)
                    xg = ln2_pool.tile([P, D], F32, tag="xg", name="xg")
                    nc.vector.tensor_mul(xg[:], xn[:], g2_bc[:])
                    fin = ln2_pool.tile([P, D], F32, tag="fin", name="fin")
                    nc.vector.tensor_add(fin[:], xg[:], be2_bc[:])
                    nc.sync.dma_start(out=out[nt * P:(nt + 1) * P, :],
                                      in_=fin[:])

            for hb in range(4):              # hidden blocks of 1024
                if hb == 0:
                    w1b = w1b0
                else:
                    w1b = [wxf_pool.tile([P, D], BF16, tag="wxf", name="wxf")
                           for _ in range(DT)]
                    for i in range(DT):
                        nc.sync.dma_start(
                            out=w1b[i],
                            in_=w1[i * P:(i + 1) * P,
                                   hb * 1024:(hb + 1) * 1024])
                w2b = [wxf_pool.tile([P, D], BF16, tag="wxf", name="wxf")
                       for _ in range(DT)]
                for i in range(DT):
                    nc.sync.dma_start(
                        out=w2b[i],
                        in_=w2[(hb * 8 + i) * P:(hb * 8 + i + 1) * P, :])
                for tc4 in range(T // 512):  # 2 token chunks of 512
                    tsl = slice(tc4 * 512, (tc4 + 1) * 512)
                    t1s = []
                    for i in range(DT):      # 8 hidden tiles in block
                        t1ps = psF1.tile([P, 512], F32, tag="t1ps",
                                         name="t1ps")
                        for din in range(DT):
                            nc.tensor.matmul(
                                t1ps[:], w1b[din][:, i * P:(i + 1) * P],
                                hT[din][:, tsl],
                                start=(din == 0), stop=(din == DT - 1))
                        t1 = t1_pool.tile([P, 512], BF16, tag="t1", name="t1")
                        nc.scalar.activation(
                            t1[:], t1ps[:], AF.Relu,
                            bias=b1_sb[:, hb * 8 + i:hb * 8 + i + 1])
                        t1s.append(t1)
                    for dout in range(DT):
                        o2ps = psF2.tile([P, 512], F32, tag="o2ps",
                                         name="o2ps")
                        for i in range(DT):
                            nc.tensor.matmul(
                                o2ps[:], w2b[i][:, dout * P:(dout + 1) * P],
                                t1s[i][:],
                                start=(i == 0), stop=(i == DT - 1))
                        if hb == 0:
                            nc.vector.tensor_copy(out2[dout][:, tsl],
                                                  o2ps[:])
                        elif hb < 3:
                            nc.vector.tensor_tensor(
                                out=out2[dout][:, tsl], in0=o2ps[:],
                                in1=out2[dout][:, tsl], op=OP.add)
                        else:
                            # last block: fold in residual + b2 -> z2T
                            tmp = ztmp_pool.tile([P, 512], F32, tag="zt",
                                                 name="zt")
                            nc.vector.tensor_tensor(
                                out=tmp[:], in0=o2ps[:],
                                in1=out2[dout][:, tsl], op=OP.add)
                            nc.vector.scalar_tensor_tensor(
                                z2T[dout][:, tsl], tmp[:],
                                biases["b2_p"][:, dout:dout + 1],
                                hT[dout][:, tsl], op0=OP.add, op1=OP.add)
                    if hb == 3:
                        # LN2 + output for this half while the other half
                        # of the FFN (or nothing) still runs
                        phase5(range(tc4 * 4, (tc4 + 1) * 4))


def _selD():
    sel = np.zeros((HD + 1, HD), dtype=np.float32)
    sel[HD, :] = 1.0
    return sel


def _pack(v, nt):
    return np.ascontiguousarray(v.reshape(nt, P).T)


def kernel(x, Wq, bq, Wk, bk, Wv, bv, Wo, bo, W1, b1, W2, b2, g1, beta1,
           g2, beta2):
    x = np.asarray(x, dtype=np.float32)
    if "nc" not in _CACHED:
        _CACHED["nc"] = _build_program()
    nc = _CACHED["nc"]

    import ml_dtypes
    bf16 = lambda a: np.ascontiguousarray(
        np.asarray(a, dtype=np.float32).astype(ml_dtypes.bfloat16))
    f32 = lambda a: np.ascontiguousarray(np.asarray(a, dtype=np.float32))
    scale = 1.0 / np.sqrt(HD)
    Wk_s = np.asarray(Wk, np.float64) * scale
    bk_s = f32(bk) * scale

    common = {
        "w1": bf16(W1), "w2": bf16(W2),
        "bo_p": _pack(f32(bo), DT), "b1_p": _pack(f32(b1), FT),
        "b2_p": _pack(f32(b2), DT),
        "g1_p": _pack(f32(g1), DT), "be1_p": _pack(f32(beta1), DT),
        "g2_d": f32(g2), "be2_d": f32(beta2),
        "ident_d": np.eye(P).astype(ml_dtypes.bfloat16),
        "ones_row_d": np.ones((1, P)).astype(ml_dtypes.bfloat16),
        "ones_col_d": np.ones((P, 1)).astype(ml_dtypes.bfloat16),
        "selD_d": _selD().astype(ml_dtypes.bfloat16),
    }
    in_maps = []
    for c in range(N_CORES):
        b, half = c // 2, c % 2
        own = x[b, half * T:(half + 1) * T]           # [1024, 1024]
        other = x[b, (1 - half) * T:(2 - half) * T]
        xT_c = np.ascontiguousarray(
            np.concatenate([own, other], axis=0).T).astype(
                ml_dtypes.bfloat16)                   # [1024, 2048]
        hsl = slice(half * 512, (half + 1) * 512)
        # [D, 512] -> [D, 4, 128] (group-major, natural order)
        wq_c = bf16(np.asarray(Wq)[:, hsl]).reshape(D, HG, P)
        wk_c = bf16(Wk_s[:, hsl]).reshape(D, HG, P)
        wv_c = bf16(np.asarray(Wv)[:, hsl]).reshape(D, HG, P)
        # wo rows: own 512 (natural), then per group g: slot0, slot1 where
        # slot s holds Wo rows of heads (s*8 + 2g, s*8+2g+1) if s != half
        # else zeros (that slot of the AllGather is this core's own data
        # for the peer's tokens -- not used here).
        Wo_np = np.asarray(Wo, np.float32)
        rows = [Wo_np[hsl]]
        for g in range(HG):
            for s in range(2):
                if s != half:
                    rows.append(Wo_np[s * 512 + g * P: s * 512 + (g + 1) * P])
                else:
                    rows.append(np.zeros((P, D), np.float32))
        wo_c = bf16(np.concatenate(rows, axis=0))     # [1536, 1024]
        bqp = _pack(f32(bq)[hsl], HG)
        bkp = _pack(bk_s[hsl], HG)
        bvp = _pack(f32(bv)[hsl], HG)
        in_maps.append({**common, "xT": np.ascontiguousarray(xT_c),
                        "wq": np.ascontiguousarray(wq_c),
                        "wk": np.ascontiguousarray(wk_c),
                        "wv": np.ascontiguousarray(wv_c),
                        "wo": wo_c,
                        "bq_p": bqp, "bk_p": bkp, "bv_p": bvp})

    trace = bool(os.environ.get("KERNEL_TRACE"))
    res = bass_utils.run_bass_kernel_spmd(
        nc, in_maps, core_ids=list(range(N_CORES)), trace=trace)
    _CACHED["last_result"] = res

    y = np.empty((4, S, D), dtype=np.float32)
    for c in range(N_CORES):
        b, half = c // 2, c % 2
        y[b, half * T:(half + 1) * T] = res.results[c]["out"]
    return y


# revision 32
# speedup vs baseline: 1.1840x; 1.1840x over previous
"""Trainium2 Bass kernel for a dense transformer layer (attention + FFN + 2 LayerNorms).

Problem shapes: x [4, 2048, 1024], d_model=1024, heads=16 (hd=64), d_ff=4096.

Sharding (8 cores): core c handles batch b = c//2.  The core PAIR (2b, 2b+1)
splits the layer two ways:
  - attention is HEAD-sharded: core half=c%2 computes Q/K/V and attention for
    its 8 heads over ALL 2048 tokens of the batch (no duplicated K/V work);
  - everything after attention (Wo, LN1, FFN, LN2) is TOKEN-sharded: each
    core owns 1024 tokens (host permutes x so own tokens are columns 0:1023).
A tiny pairwise AllGather per 2-head group moves the normalized attention
context for the peer's token half across the pair (4 x 256KB, overlapped with
the remaining attention groups).  The peer's contribution enters Wo as extra
stationary row-blocks whose unused slot rows are zeroed host-side, so no
per-core control flow is needed (pure SPMD).

Layout: activations feature-major (features on partitions, tokens free).
Scores are computed per head with contraction 64 (S^T = K_h Q_h^T, keys on
partitions), exp on the scalar engine over [128,1024] PSUM tiles, ctx via
V_aug (ones column -> softmax denominator for free).  QKV projection of group
g+1 overlaps the scalar-bound attention of group g on the PE.
"""

import os
import numpy as np

import concourse.bass as bass
import concourse.tile as tile
from concourse import bacc, mybir
from concourse import bass_utils

BF16 = mybir.dt.bfloat16
F32 = mybir.dt.float32
AF = mybir.ActivationFunctionType
OP = mybir.AluOpType

D = 1024          # d_model
S = 2048          # full sequence per batch
T = 1024          # tokens owned per core (post-attention)
H = 16            # total heads
HG = 4            # local 2-head groups per core (8 heads)
HD = 64           # head dim
F = 4096          # ffn hidden
P = 128
DT = D // P       # 8 feature tiles
KT = S // P       # 16 key-token tiles
FT = F // P       # 32 hidden tiles
N_CORES = 8
EPS = 1e-5
GROUPS = [[0, 1], [2, 3], [4, 5], [6, 7]]

_CACHED = {}


def _build_program():
    nc = bacc.Bacc("TRN2", target_bir_lowering=False, debug=False,
                   num_devices=N_CORES)

    tens = {}

    def di(name, shape, dtype=BF16):
        tens[name] = nc.dram_tensor(name, shape, dtype, kind="ExternalInput")

    di("xT", [D, S])
    di("wq", [D, HG, P]); di("wk", [D, HG, P]); di("wv", [D, HG, P])
    di("wo", [12 * P, D])
    di("w1", [D, F]); di("w2", [F, D])
    for nm in ["bq_p", "bk_p", "bv_p"]:
        di(nm, [P, HG], F32)
    for nm in ["bo_p", "b2_p", "g1_p", "be1_p"]:
        di(nm, [P, DT], F32)
    di("b1_p", [P, FT], F32)
    di("g2_d", [D], F32); di("be2_d", [D], F32)
    di("ident_d", [P, P]); di("ones_row_d", [1, P]); di("ones_col_d", [P, 1])
    di("selD_d", [HD + 1, HD])
    tens["out"] = nc.dram_tensor("out", [T, D], F32, kind="ExternalOutput")

    with tile.TileContext(nc) as tc:
        _trace_kernel(nc, tc, tens)
    nc.compile()
    return nc


def _trace_kernel(nc, tc, t):
    xT, wq, wk, wv, wo, w1, w2 = (t["xT"], t["wq"], t["wk"], t["wv"], t["wo"],
                                  t["w1"], t["w2"])
    out = t["out"]

    from contextlib import ExitStack
    es = ExitStack()
    with es:
        dram = es.enter_context(tc.tile_pool(name="dram", bufs=1, space="DRAM"))

        const = es.enter_context(tc.tile_pool(name="const", bufs=1))
        ident = const.tile([P, P], BF16, tag="ident", name="ident")
        nc.sync.dma_start(out=ident, in_=t["ident_d"][:, :])
        ones_row = const.tile([1, P], BF16, tag="onesr", name="onesr")
        nc.sync.dma_start(out=ones_row, in_=t["ones_row_d"][:, :])
        ones_col = const.tile([P, 1], BF16, tag="onesc", name="onesc")
        nc.sync.dma_start(out=ones_col, in_=t["ones_col_d"][:, :])
        selD = const.tile([HD + 1, HD], BF16, tag="selD", name="selD")
        nc.sync.dma_start(out=selD, in_=t["selD_d"][:, :])
        biases = {}
        for name in ["bq_p", "bk_p", "bv_p"]:
            bt = const.tile([P, HG], F32, tag=name)
            nc.sync.dma_start(out=bt, in_=t[name][:, :])
            biases[name] = bt
        for name in ["bo_p", "b2_p", "g1_p", "be1_p"]:
            bt = const.tile([P, DT], F32, tag=name)
            nc.sync.dma_start(out=bt, in_=t[name][:, :])
            biases[name] = bt
        eps_sb = const.tile([P, 1], F32, tag="eps", name="eps")
        nc.vector.memset(eps_sb[:], EPS)
        b1_sb = const.tile([P, FT], F32, tag="b1", name="b1")
        nc.sync.dma_start(out=b1_sb, in_=t["b1_p"][:, :])

        # long-lived pools (outlive phase W into the FFN) must open before
        # the phase-F/W pools below them on the allocation stack
        hT_pool = es.enter_context(tc.tile_pool(name="hT", bufs=1))
        hT = [hT_pool.tile([P, T], BF16, tag=f"hT{i}", name=f"hT{i}")
              for i in range(DT)]
        wx_pool = es.enter_context(tc.tile_pool(name="wx", bufs=12))
        w1p_pool = es.enter_context(tc.tile_pool(name="w1p", bufs=8))

        # pools that die after phase W (freed before the FFN needs SBUF)
        es_fw = ExitStack()
        # K weights first (small) so the first projection chain isn't
        # queued behind the 4MB x load, then x, then Q/V weights
        wqkv_pool = es_fw.enter_context(tc.tile_pool(name="wqkv", bufs=1))
        wsl = {}

        def load_w(nm, wd):
            tiles = []
            for din in range(DT):
                wt = wqkv_pool.tile([P, HG, P], BF16, tag=f"w{nm}{din}",
                                    name=f"w{nm}{din}")
                nc.sync.dma_start(out=wt, in_=wd[din * P:(din + 1) * P, :, :])
                tiles.append(wt)
            wsl[nm] = tiles

        load_w("k", wk)
        xsb_pool = es_fw.enter_context(tc.tile_pool(name="xsb", bufs=1))
        xsb = []
        for dt_ in range(DT):
            xt_ = xsb_pool.tile([P, S], BF16, tag=f"x{dt_}", name=f"x{dt_}")
            nc.sync.dma_start(out=xt_, in_=xT[dt_ * P:(dt_ + 1) * P, :])
            xsb.append(xt_)
        load_w("v", wv)
        load_w("q", wq)

        # normalized ctx per group (own token half only), persists until Wo
        ctxn_pool = es_fw.enter_context(tc.tile_pool(name="ctxn", bufs=1))
        ctxn = [ctxn_pool.tile([P, T], BF16, tag=f"cn{g}", name=f"cn{g}")
                for g in range(HG)]

        # AllGather bounce buffers (HBM)
        agin = [dram.tile([P, T], BF16, tag=f"agi{g}", name=f"agi{g}")
                for g in range(HG)]
        agout = [dram.tile([2, P, T], BF16, tag=f"ago{g}", name=f"ago{g}")
                 for g in range(HG)]

        # =============== Phase F: fused QKV + attention per group ========
        with tc.tile_pool(name="kbuf", bufs=2) as kbuf_pool, \
             tc.tile_pool(name="qbuf", bufs=2) as qbuf_pool, \
             tc.tile_pool(name="vbuf", bufs=2) as vbuf_pool, \
             tc.tile_pool(name="vtmp", bufs=2) as vtmp_pool, \
             tc.tile_pool(name="ctx", bufs=2) as ctx_pool, \
             tc.tile_pool(name="pbuf", bufs=4) as pbuf_pool, \
             tc.tile_pool(name="rcp", bufs=2) as rcp_pool, \
             tc.tile_pool(name="psS", bufs=2, space="PSUM") as psS, \
             tc.tile_pool(name="psC", bufs=2, space="PSUM") as psC, \
             tc.tile_pool(name="psA", bufs=1, space="PSUM") as psA, \
             tc.tile_pool(name="psX", bufs=1, space="PSUM") as psX:

            for g in range(HG):
                # ---- projections for group g (2 heads = 128 features) ----
                ksb = kbuf_pool.tile([P, S], BF16, tag="kb", name="kb")
                # Q staged zero-padded per head: sibling head rows zero so
                # the full [128,128] K stationary tiles cancel them
                qz = [qbuf_pool.tile([P, S], BF16, tag=f"qz{hh}",
                                     name=f"qz{hh}") for hh in range(2)]
                for hh in range(2):
                    nc.vector.memset(qz[hh][:], 0.0)
                vsb = vbuf_pool.tile([P, KT, 2, HD + 1], BF16, tag="vb",
                                     name="vb")

                def proj(wtiles, bias, dest_cb, g=g):
                    for ch in range(S // 512):
                        ps = psA.tile([P, 512], F32, tag="psA", name="psA")
                        for din in range(DT):
                            nc.tensor.matmul(
                                ps[:], wtiles[din][:, g, :],
                                xsb[din][:, ch * 512:(ch + 1) * 512],
                                start=(din == 0), stop=(din == DT - 1))
                        dest_cb(ps, ch)

                def k_evict(ps, ch, ksb=ksb, g=g):
                    nc.vector.tensor_scalar(
                        out=ksb[:, ch * 512:(ch + 1) * 512], in0=ps[:],
                        scalar1=biases["bk_p"][:, g:g + 1], scalar2=None,
                        op0=OP.add)

                def q_evict(ps, ch, qz=qz, g=g):
                    for hh in range(2):
                        r0 = hh * HD
                        nc.vector.tensor_scalar(
                            out=qz[hh][r0:r0 + HD, ch * 512:(ch + 1) * 512],
                            in0=ps[r0:r0 + HD, :],
                            scalar1=biases["bq_p"][r0:r0 + HD, g:g + 1],
                            scalar2=None, op0=OP.add)

                def v_evict(ps, ch, vsb=vsb, g=g):
                    vt = vtmp_pool.tile([P, 512], BF16, tag="vt", name="vt")
                    nc.vector.tensor_scalar(
                        out=vt[:], in0=ps[:],
                        scalar1=biases["bv_p"][:, g:g + 1], scalar2=None,
                        op0=OP.add)
                    for hh in range(2):
                        idsl = ident[hh * HD:(hh + 1) * HD,
                                     hh * HD:(hh + 1) * HD]
                        for st in range(4):
                            pt = psX.tile([P, HD], BF16, tag="psX",
                                          name="psX")
                            nc.tensor.transpose(
                                pt[:],
                                vt[hh * HD:(hh + 1) * HD,
                                   st * P:(st + 1) * P], idsl)
                            nc.vector.tensor_copy(
                                vsb[:, ch * 4 + st, hh, 0:HD], pt[:])

                proj(wsl["k"], "bk_p", k_evict)
                proj(wsl["v"], "bv_p", v_evict)
                nc.vector.memset(vsb[:, :, :, HD:HD + 1], 1.0)
                proj(wsl["q"], "bq_p", q_evict)

                # ---- attention for the 2 heads over all 2048 queries ----
                # pass 0: own token half -> ctxn[g]; pass 1: peer -> send
                send = ctx_pool.tile([P, T], BF16, tag="send", name="send")
                for ps_ in (1, 0):       # peer tokens first -> AG launches
                    p0 = ps_ * 1024      # earlier and overlaps more compute
                    for hh in range(2):
                        r0 = hh * HD
                        cpss = [psC.tile([HD + 1, 512], F32, tag="cps",
                                         name="cps") for _ in range(2)]
                        for jp in range(KT // 2):
                            j0, j1 = 2 * jp, 2 * jp + 1
                            for qc in range(2):
                                c0 = p0 + qc * 512
                                sps = psS.tile([P, 2, 512], F32, tag="sps",
                                               name="sps")
                                nc.tensor.matmul(
                                    sps[:, 0, :],
                                    ksb[:, j0 * P:(j0 + 1) * P],
                                    qz[hh][:, c0:c0 + 512],
                                    start=True, stop=True)
                                nc.tensor.matmul(
                                    sps[:, 1, :],
                                    ksb[:, j1 * P:(j1 + 1) * P],
                                    qz[hh][:, c0:c0 + 512],
                                    start=True, stop=True)
                                pT = pbuf_pool.tile([P, 2, 512], BF16,
                                                    tag="pT", name="pT")
                                nc.scalar.activation(pT[:], sps[:], AF.Exp)
                                nc.tensor.matmul(
                                    cpss[qc][:], vsb[:, j0, hh, :],
                                    pT[:, 0, :],
                                    start=(jp == 0), stop=False)
                                nc.tensor.matmul(
                                    cpss[qc][:], vsb[:, j1, hh, :],
                                    pT[:, 1, :],
                                    start=False, stop=(jp == KT // 2 - 1))
                        # evict + normalize by softmax denominator (row 64)
                        dest = ctxn[g] if ps_ == 0 else send
                        for qc in range(2):
                            ctx_sb = ctx_pool.tile([HD + 1, 512], BF16,
                                                   tag="cs", name="cs")
                            nc.vector.tensor_copy(ctx_sb[:], cpss[qc][:])
                            dn = psA.tile([P, 512], F32, tag="psA",
                                          name="psA")
                            nc.tensor.matmul(
                                dn[0:HD, :], selD[:, :], ctx_sb[:],
                                start=True, stop=True)
                            rc = rcp_pool.tile([HD, 512], F32, tag="rc",
                                               name="rc")
                            nc.vector.reciprocal(rc[:], dn[0:HD, :])
                            nc.vector.tensor_tensor(
                                out=dest[r0:r0 + HD,
                                         qc * 512:(qc + 1) * 512],
                                in0=ctx_sb[0:HD, :],
                                in1=rc[:], op=OP.mult)

                # ---- ship peer's token half to the pair core ------------
                nc.gpsimd.dma_start(agin[g][:, :], send[:])
                nc.gpsimd.collective_compute(
                    "AllGather", OP.bypass, replica_groups=GROUPS,
                    ins=[agin[g][:, :].opt()],
                    outs=[agout[g][:, :, :].opt()])

            # Fence: a 5th tiny AllGather.  The tile framework makes each
            # collective's input writer wait until all PRIOR collectives'
            # data has ARRIVED (the trigger instruction itself completes
            # early), so the fence writer + gpsimd program order make the
            # agout readbacks below race-free.
            fence_sb = const.tile([P, 8], BF16, tag="fsb", name="fsb")
            nc.vector.memset(fence_sb[:], 0.0)
            fence_in = dram.tile([P, 8], BF16, tag="fin", name="fin")
            fence_out = dram.tile([2, P, 8], BF16, tag="fout", name="fout")
            nc.gpsimd.dma_start(fence_in[:, :], fence_sb[:])
            nc.gpsimd.collective_compute(
                "AllGather", OP.bypass, replica_groups=GROUPS,
                ins=[fence_in[:, :].opt()],
                outs=[fence_out[:, :, :].opt()])

        # =============== Phase W: Wo + residual + LN1 ===================
        with tc.tile_pool(name="asb", bufs=1) as asb_pool, \
             tc.tile_pool(name="zT", bufs=1) as zT_pool, \
             tc.tile_pool(name="ln1", bufs=2) as ln1_pool, \
             tc.tile_pool(name="psW", bufs=3, space="PSUM") as psW, \
             tc.tile_pool(name="psStat", bufs=1, space="PSUM") as psStat, \
             tc.tile_pool(name="psBc", bufs=1, space="PSUM") as psBc:
            wo_sb = [wx_pool.tile([P, D], BF16, tag="wx", name="wx")
                     for _ in range(12)]
            for i in range(12):
                nc.sync.dma_start(out=wo_sb[i], in_=wo[i * P:(i + 1) * P, :])
            # FFN hb=0 W1 preload (overlaps W phase)
            w1b0 = [w1p_pool.tile([P, D], BF16, tag="w1p", name="w1p")
                    for _ in range(DT)]
            for i in range(DT):
                nc.sync.dma_start(out=w1b0[i], in_=w1[i * P:(i + 1) * P,
                                                      0:1024])

            # peer ctx contributions from the AllGathers (both slots; the
            # useless slot's wo rows are zero)
            asb = []
            for g in range(HG):
                for s in range(2):
                    a = asb_pool.tile([P, T], BF16, tag=f"a{g}{s}",
                                      name=f"a{g}{s}")
                    # gpsimd queue: executes after the fence writer above,
                    # i.e. after every AllGather's data has arrived
                    nc.gpsimd.dma_start(a[:], agout[g][s, :, :])
                    asb.append(a)
            # moving operands for Wo in wo_sb row order:
            #   rows 0:512   -> own ctxn groups 0..3 (own tokens = cols 0:T)
            #   rows 512:1536 -> agout g0 s0, g0 s1, g1 s0, ... (g3 last so
            #   the last AllGather's wait overlaps the first 10 matmuls)
            movs = [ctxn[g][:] for g in range(HG)] + asb

            zT = [zT_pool.tile([P, T], BF16, tag=f"zT{i}", name=f"zT{i}")
                  for i in range(DT)]
            for ch in range(T // 512):
                for dout in range(DT):
                    ps = psW.tile([P, 512], F32, tag="psW", name="psW")
                    for din in range(12):
                        nc.tensor.matmul(
                            ps[:], wo_sb[din][:, dout * P:(dout + 1) * P],
                            movs[din][:, ch * 512:(ch + 1) * 512],
                            start=(din == 0), stop=(din == 11))
                    # z = attn_out + bo + x_resid
                    nc.vector.scalar_tensor_tensor(
                        zT[dout][:, ch * 512:(ch + 1) * 512], ps[:],
                        biases["bo_p"][:, dout:dout + 1],
                        xsb[dout][:, ch * 512:(ch + 1) * 512],
                        op0=OP.add, op1=OP.add)

            # ---- LN1 (feature-major; stats over partitions via PE) -----
            for ch in range(T // 512):
                sl = slice(ch * 512, (ch + 1) * 512)
                sum_ps = psStat.tile([1, 512], F32, tag="s", name="s")
                sq_ps = psStat.tile([1, 512], F32, tag="q", name="q")
                for dt_ in range(DT):
                    zsq = ln1_pool.tile([P, 512], BF16, tag="zsq",
                                        name="zsq")
                    nc.scalar.activation(zsq[:], zT[dt_][:, sl], AF.Square)
                    nc.tensor.matmul(sum_ps[:], ones_col[:], zT[dt_][:, sl],
                                     start=(dt_ == 0), stop=(dt_ == DT - 1))
                    nc.tensor.matmul(sq_ps[:], ones_col[:], zsq[:],
                                     start=(dt_ == 0), stop=(dt_ == DT - 1))
                mean = ln1_pool.tile([1, 512], F32, tag="mean", name="mean")
                nc.scalar.mul(mean[:], sum_ps[:], 1.0 / D)
                msq = ln1_pool.tile([1, 512], F32, tag="msq", name="msq")
                nc.scalar.mul(msq[:], sq_ps[:], 1.0 / D)
                m2 = ln1_pool.tile([1, 512], F32, tag="m2", name="m2")
                nc.vector.tensor_mul(m2[:], mean[:], mean[:])
                var = ln1_pool.tile([1, 512], F32, tag="var", name="var")
                nc.vector.tensor_sub(var[:], msq[:], m2[:])
                std = ln1_pool.tile([1, 512], F32, tag="std", name="std")
                nc.scalar.activation(std[:], var[:], AF.Sqrt,
                                     bias=eps_sb[0:1, :])
                rstd = ln1_pool.tile([1, 512], F32, tag="rstd", name="rstd")
                nc.vector.reciprocal(rstd[:], std[:])
                mean_r = ln1_pool.tile([1, 512], BF16, tag="meanr",
                                       name="meanr")
                nc.vector.tensor_copy(mean_r[:], mean[:])
                rstd_r = ln1_pool.tile([1, 512], BF16, tag="rstdr",
                                       name="rstdr")
                nc.vector.tensor_copy(rstd_r[:], rstd[:])
                bm_ps = psBc.tile([P, 512], F32, tag="bm", name="bm")
                nc.tensor.matmul(bm_ps[:], ones_row[:], mean_r[:],
                                 start=True, stop=True)
                br_ps = psBc.tile([P, 512], F32, tag="br", name="br")
                nc.tensor.matmul(br_ps[:], ones_row[:], rstd_r[:],
                                 start=True, stop=True)
                bm = ln1_pool.tile([P, 512], F32, tag="bm_sb", name="bm_sb")
                nc.scalar.copy(bm[:], bm_ps[:])
                br = ln1_pool.tile([P, 512], F32, tag="br_sb", name="br_sb")
                nc.scalar.copy(br[:], br_ps[:])
                for dt_ in range(DT):
                    tmp = ln1_pool.tile([P, 512], F32, tag="n1", name="n1")
                    nc.vector.scalar_tensor_tensor(
                        tmp[:], zT[dt_][:, sl],
                        1.0, bm[:], op0=OP.mult, op1=OP.subtract)
                    tmp2 = ln1_pool.tile([P, 512], F32, tag="n2", name="n2")
                    nc.vector.scalar_tensor_tensor(
                        tmp2[:], tmp[:],
                        biases["g1_p"][:, dt_:dt_ + 1], br[:],
                        op0=OP.mult, op1=OP.mult)
                    nc.scalar.activation(
                        hT[dt_][:, sl], tmp2[:], AF.Identity,
                        bias=biases["be1_p"][:, dt_:dt_ + 1])

        es_fw.close()   # free x / QKV weights / ctx SBUF before the FFN

        # =============== Phase 4: FFN + residual + fused LN2/out ========
        with tc.tile_pool(name="wxf", bufs=20) as wxf_pool, \
             tc.tile_pool(name="z2T", bufs=1) as z2T_pool, \
             tc.tile_pool(name="t1", bufs=12) as t1_pool, \
             tc.tile_pool(name="ztmp", bufs=2) as ztmp_pool, \
             tc.tile_pool(name="o2", bufs=1) as o2_pool, \
             tc.tile_pool(name="tm", bufs=2) as tm_pool, \
             tc.tile_pool(name="lnc", bufs=1) as lnc_pool, \
             tc.tile_pool(name="ln2", bufs=2) as ln2_pool, \
             tc.tile_pool(name="psF1", bufs=3, space="PSUM") as psF1, \
             tc.tile_pool(name="psF2", bufs=3, space="PSUM") as psF2, \
             tc.tile_pool(name="psT5", bufs=2, space="PSUM") as psT5:
            z2T = [z2T_pool.tile([P, T], BF16, tag=f"z2T{i}", name=f"z2T{i}")
                   for i in range(DT)]
            out2 = [o2_pool.tile([P, T], F32, tag=f"o2{i}", name=f"o2{i}")
                    for i in range(DT)]
            g2_bc = lnc_pool.tile([P, D], F32, tag="g2bc", name="g2bc")
            nc.sync.dma_start(out=g2_bc, in_=bass.AP(
                tensor=t["g2_d"], offset=0, ap=[[0, P], [1, D]]))
            be2_bc = lnc_pool.tile([P, D], F32, tag="be2bc", name="be2bc")
            nc.sync.dma_start(out=be2_bc, in_=bass.AP(
                tensor=t["be2_d"], offset=0, ap=[[0, P], [1, D]]))

            def phase5(nts):
                for nt in nts:
                    z2 = tm_pool.tile([P, D], F32, tag="z2tm", name="z2tm")
                    for dt_ in range(DT):
                        pt = psT5.tile([P, P], BF16, tag="psT5", name="psT5")
                        nc.tensor.transpose(
                            pt[:], z2T[dt_][:, nt * P:(nt + 1) * P],
                            ident[:])
                        nc.scalar.copy(z2[:, dt_ * P:(dt_ + 1) * P], pt[:])
                    stats = ln2_pool.tile([P, 2, 6], F32, tag="st", name="st")
                    for gg in range(2):
                        nc.vector.bn_stats(out=stats[:, gg, :],
                                           in_=z2[:, gg * 512:(gg + 1) * 512])
                    mv = ln2_pool.tile([P, 2], F32, tag="mv", name="mv")
                    nc.vector.bn_aggr(out=mv[:], in_=stats[:])
                    std = ln2_pool.tile([P, 1], F32, tag="std2", name="std2")
                    nc.scalar.activation(std[:], mv[:, 1:2], AF.Sqrt,
                                         bias=eps_sb[:])
                    rstd = ln2_pool.tile([P, 1], F32, tag="rstd2",
                                         name="rstd2")
                    nc.vector.reciprocal(rstd[:], std[:])
                    xn = ln2_pool.tile([P, D], F32, tag="xn", name="xn")
                    nc.vector.tensor_scalar(
                        out=xn[:], in0=z2[:], scalar1=mv[:, 0:1],
                        scalar2=rstd[:], op0=OP.subtract, op1=OP.mult)
                    xg = ln2_pool.tile([P, D], F32, tag="xg", name="xg")
                    nc.vector.tensor_mul(xg[:], xn[:], g2_bc[:])
                    fin = ln2_pool.tile([P, D], F32, tag="fin", name="fin")
                    nc.vector.tensor_add(fin[:], xg[:], be2_bc[:])
                    nc.sync.dma_start(out=out[nt * P:(nt + 1) * P, :],
                                      in_=fin[:])

            for hb in range(4):              # hidden blocks of 1024
                if hb == 0:
                    w1b = w1b0
                else:
                    w1b = [wxf_pool.tile([P, D], BF16, tag="wxf", name="wxf")
                           for _ in range(DT)]
                    for i in range(DT):
                        nc.sync.dma_start(
                            out=w1b[i],
                            in_=w1[i * P:(i + 1) * P,
                                   hb * 1024:(hb + 1) * 1024])
                w2b = [wxf_pool.tile([P, D], BF16, tag="wxf", name="wxf")
                       for _ in range(DT)]
                for i in range(DT):
                    nc.sync.dma_start(
                        out=w2b[i],
                        in_=w2[(hb * 8 + i) * P:(hb * 8 + i + 1) * P, :])
                for tc4 in range(T // 512):  # 2 token chunks of 512
                    tsl = slice(tc4 * 512, (tc4 + 1) * 512)
                    t1s = []
                    for i in range(DT):      # 8 hidden tiles in block
                        t1ps = psF1.tile([P, 512], F32, tag="t1ps",
                                         name="t1ps")
                        for din in range(DT):
                            nc.tensor.matmul(
                                t1ps[:], w1b[din][:, i * P:(i + 1) * P],
                                hT[din][:, tsl],
                                start=(din == 0), stop=(din == DT - 1))
                        t1 = t1_pool.tile([P, 512], BF16, tag="t1", name="t1")
                        nc.scalar.activation(
                            t1[:], t1ps[:], AF.Relu,
                            bias=b1_sb[:, hb * 8 + i:hb * 8 + i + 1])
                        t1s.append(t1)
                    for dout in range(DT):
                        o2ps = psF2.tile([P, 512], F32, tag="o2ps",
                                         name="o2ps")
                        for i in range(DT):
                            nc.tensor.matmul(
                                o2ps[:], w2b[i][:, dout * P:(dout + 1) * P],
                                t1s[i][:],
                                start=(i == 0), stop=(i == DT - 1))
                        if hb == 0:
                            nc.vector.tensor_copy(out2[dout][:, tsl],
                                                  o2ps[:])
                        elif hb < 3:
                            nc.vector.tensor_tensor(
                                out=out2[dout][:, tsl], in0=o2ps[:],
                                in1=out2[dout][:, tsl], op=OP.add)
                        else:
                            # last block: fold in residual + b2 -> z2T
                            tmp = ztmp_pool.tile([P, 512], F32, tag="zt",
                                                 name="zt")
                            nc.vector.tensor_tensor(
                                out=tmp[:], in0=o2ps[:],
                                in1=out2[dout][:, tsl], op=OP.add)
                            nc.vector.scalar_tensor_tensor(
                                z2T[dout][:, tsl], tmp[:],
                                biases["b2_p"][:, dout:dout + 1],
                                hT[dout][:, tsl], op0=OP.add, op1=OP.add)
                    if hb == 3:
                        # LN2 + output for this half while the other half
                        # of the FFN (or nothing) still runs
                        phase5(range(tc4 * 4, (tc4 + 1) * 4))


def _selD():
    sel = np.zeros((HD + 1, HD), dtype=np.float32)
    sel[HD, :] = 1.0
    return sel


def _pack(v, nt):
    return np.ascontiguousarray(v.reshape(nt, P).T)


def kernel(x, Wq, bq, Wk, bk, Wv, bv, Wo, bo, W1, b1, W2, b2, g1, beta1,
           g2, beta2):
    x = np.asarray(x, dtype=np.float32)
    if "nc" not in _CACHED:
        _CACHED["nc"] = _build_program()
    nc = _CACHED["nc"]

    import ml_dtypes
    bf16 = lambda a: np.ascontiguousarray(
        np.asarray(a, dtype=np.float32).astype(ml_dtypes.bfloat16))
    f32 = lambda a: np.ascontiguousarray(np.asarray(a, dtype=np.float32))
    scale = 1.0 / np.sqrt(HD)
    Wk_s = np.asarray(Wk, np.float64) * scale
    bk_s = f32(bk) * scale

    common = {
        "w1": bf16(W1), "w2": bf16(W2),
        "bo_p": _pack(f32(bo), DT), "b1_p": _pack(f32(b1), FT),
        "b2_p": _pack(f32(b2), DT),
        "g1_p": _pack(f32(g1), DT), "be1_p": _pack(f32(beta1), DT),
        "g2_d": f32(g2), "be2_d": f32(beta2),
        "ident_d": np.eye(P).astype(ml_dtypes.bfloat16),
        "ones_row_d": np.ones((1, P)).astype(ml_dtypes.bfloat16),
        "ones_col_d": np.ones((P, 1)).astype(ml_dtypes.bfloat16),
        "selD_d": _selD().astype(ml_dtypes.bfloat16),
    }
    in_maps = []
    for c in range(N_CORES):
        b, half = c // 2, c % 2
        own = x[b, half * T:(half + 1) * T]           # [1024, 1024]
        other = x[b, (1 - half) * T:(2 - half) * T]
        xT_c = np.ascontiguousarray(
            np.concatenate([own, other], axis=0).T).astype(
                ml_dtypes.bfloat16)                   # [1024, 2048]
        hsl = slice(half * 512, (half + 1) * 512)
        # [D, 512] -> [D, 4, 128] (group-major, natural order)
        wq_c = bf16(np.asarray(Wq)[:, hsl]).reshape(D, HG, P)
        wk_c = bf16(Wk_s[:, hsl]).reshape(D, HG, P)
        wv_c = bf16(np.asarray(Wv)[:, hsl]).reshape(D, HG, P)
        # wo rows: own 512 (natural), then per group g: slot0, slot1 where
        # slot s holds Wo rows of heads (s*8 + 2g, s*8+2g+1) if s != half
        # else zeros (that slot of the AllGather is this core's own data
        # for the peer's tokens -- not used here).
        Wo_np = np.asarray(Wo, np.float32)
        rows = [Wo_np[hsl]]
        for g in range(HG):
            for s in range(2):
                if s != half:
                    rows.append(Wo_np[s * 512 + g * P: s * 512 + (g + 1) * P])
                else:
                    rows.append(np.zeros((P, D), np.float32))
        wo_c = bf16(np.concatenate(rows, axis=0))     # [1536, 1024]
        bqp = _pack(f32(bq)[hsl], HG)
        bkp = _pack(bk_s[hsl], HG)
        bvp = _pack(f32(bv)[hsl], HG)
        in_maps.append({**common, "xT": np.ascontiguousarray(xT_c),
                        "wq": np.ascontiguousarray(wq_c),
                        "wk": np.ascontiguousarray(wk_c),
                        "wv": np.ascontiguousarray(wv_c),
                        "wo": wo_c,
                        "bq_p": bqp, "bk_p": bkp, "bv_p": bvp})

    trace = bool(os.environ.get("KERNEL_TRACE"))
    res = bass_utils.run_bass_kernel_spmd(
        nc, in_maps, core_ids=list(range(N_CORES)), trace=trace)
    _CACHED["last_result"] = res

    y = np.empty((4, S, D), dtype=np.float32)
    for c in range(N_CORES):
        b, half = c // 2, c % 2
        y[b, half * T:(half + 1) * T] = res.results[c]["out"]
    return y


# revision 33
# speedup vs baseline: 1.2151x; 1.0263x over previous
"""Trainium2 Bass kernel for a dense transformer layer (attention + FFN + 2 LayerNorms).

Problem shapes: x [4, 2048, 1024], d_model=1024, heads=16 (hd=64), d_ff=4096.

Sharding (8 cores): core c handles batch b = c//2.  The core PAIR (2b, 2b+1)
splits the layer two ways:
  - attention is HEAD-sharded: core half=c%2 computes Q/K/V and attention for
    its 8 heads over ALL 2048 tokens of the batch (no duplicated K/V work);
  - everything after attention (Wo, LN1, FFN, LN2) is TOKEN-sharded: each
    core owns 1024 tokens (host permutes x so own tokens are columns 0:1023).
A tiny pairwise AllGather per 2-head group moves the normalized attention
context for the peer's token half across the pair (4 x 256KB, overlapped with
the remaining attention groups).  The peer's contribution enters Wo as extra
stationary row-blocks whose unused slot rows are zeroed host-side, so no
per-core control flow is needed (pure SPMD).

Layout: activations feature-major (features on partitions, tokens free).
Scores are computed per head with contraction 64 (S^T = K_h Q_h^T, keys on
partitions), exp on the scalar engine over [128,1024] PSUM tiles, ctx via
V_aug (ones column -> softmax denominator for free).  QKV projection of group
g+1 overlaps the scalar-bound attention of group g on the PE.
"""

import os
import numpy as np

import concourse.bass as bass
import concourse.tile as tile
from concourse import bacc, mybir
from concourse import bass_utils

BF16 = mybir.dt.bfloat16
F32 = mybir.dt.float32
AF = mybir.ActivationFunctionType
OP = mybir.AluOpType

D = 1024          # d_model
S = 2048          # full sequence per batch
T = 1024          # tokens owned per core (post-attention)
H = 16            # total heads
HG = 4            # local 2-head groups per core (8 heads)
HD = 64           # head dim
F = 4096          # ffn hidden
P = 128
DT = D // P       # 8 feature tiles
KT = S // P       # 16 key-token tiles
FT = F // P       # 32 hidden tiles
N_CORES = 8
EPS = 1e-5
GROUPS = [[0, 1], [2, 3], [4, 5], [6, 7]]

_CACHED = {}


def _build_program():
    nc = bacc.Bacc("TRN2", target_bir_lowering=False, debug=False,
                   num_devices=N_CORES)

    tens = {}

    def di(name, shape, dtype=BF16):
        tens[name] = nc.dram_tensor(name, shape, dtype, kind="ExternalInput")

    di("xT", [D, S])
    di("wq", [D, HG, P]); di("wk", [D, HG, P]); di("wv", [D, HG, P])
    di("wo", [12 * P, D])
    di("w1", [D, F]); di("w2", [F, D])
    for nm in ["bq_p", "bk_p", "bv_p"]:
        di(nm, [P, HG], F32)
    for nm in ["bo_p", "b2_p", "g1_p", "be1_p"]:
        di(nm, [P, DT], F32)
    di("b1_p", [P, FT], F32)
    di("g2_d", [D], F32); di("be2_d", [D], F32)
    di("ident_d", [P, P]); di("ones_row_d", [1, P]); di("ones_col_d", [P, 1])
    di("selD_d", [HD + 1, HD])
    tens["out"] = nc.dram_tensor("out", [T, D], F32, kind="ExternalOutput")

    with tile.TileContext(nc) as tc:
        _trace_kernel(nc, tc, tens)
    nc.compile()
    return nc


def _trace_kernel(nc, tc, t):
    xT, wq, wk, wv, wo, w1, w2 = (t["xT"], t["wq"], t["wk"], t["wv"], t["wo"],
                                  t["w1"], t["w2"])
    out = t["out"]

    from contextlib import ExitStack
    es = ExitStack()
    with es:
        dram = es.enter_context(tc.tile_pool(name="dram", bufs=1, space="DRAM"))

        const = es.enter_context(tc.tile_pool(name="const", bufs=1))
        ident = const.tile([P, P], BF16, tag="ident", name="ident")
        nc.sync.dma_start(out=ident, in_=t["ident_d"][:, :])
        ones_row = const.tile([1, P], BF16, tag="onesr", name="onesr")
        nc.sync.dma_start(out=ones_row, in_=t["ones_row_d"][:, :])
        ones_col = const.tile([P, 1], BF16, tag="onesc", name="onesc")
        nc.sync.dma_start(out=ones_col, in_=t["ones_col_d"][:, :])
        selD = const.tile([HD + 1, HD], BF16, tag="selD", name="selD")
        nc.sync.dma_start(out=selD, in_=t["selD_d"][:, :])
        biases = {}
        for name in ["bq_p", "bk_p", "bv_p"]:
            bt = const.tile([P, HG], F32, tag=name)
            nc.sync.dma_start(out=bt, in_=t[name][:, :])
            biases[name] = bt
        for name in ["bo_p", "b2_p", "g1_p", "be1_p"]:
            bt = const.tile([P, DT], F32, tag=name)
            nc.sync.dma_start(out=bt, in_=t[name][:, :])
            biases[name] = bt
        eps_sb = const.tile([P, 1], F32, tag="eps", name="eps")
        nc.vector.memset(eps_sb[:], EPS)
        b1_sb = const.tile([P, FT], F32, tag="b1", name="b1")
        nc.sync.dma_start(out=b1_sb, in_=t["b1_p"][:, :])

        # long-lived pools (outlive phase W into the FFN) must open before
        # the phase-F/W pools below them on the allocation stack
        hT_pool = es.enter_context(tc.tile_pool(name="hT", bufs=1))
        hT = [hT_pool.tile([P, T], BF16, tag=f"hT{i}", name=f"hT{i}")
              for i in range(DT)]
        wx_pool = es.enter_context(tc.tile_pool(name="wx", bufs=12))
        w1p_pool = es.enter_context(tc.tile_pool(name="w1p", bufs=8))

        # pools that die after phase W (freed before the FFN needs SBUF)
        es_fw = ExitStack()
        # K weights first (small) so the first projection chain isn't
        # queued behind the 4MB x load, then x, then Q/V weights
        wqkv_pool = es_fw.enter_context(tc.tile_pool(name="wqkv", bufs=1))
        wsl = {}

        def load_w(nm, wd):
            tiles = []
            for din in range(DT):
                wt = wqkv_pool.tile([P, HG, P], BF16, tag=f"w{nm}{din}",
                                    name=f"w{nm}{din}")
                nc.sync.dma_start(out=wt, in_=wd[din * P:(din + 1) * P, :, :])
                tiles.append(wt)
            wsl[nm] = tiles

        load_w("k", wk)
        xsb_pool = es_fw.enter_context(tc.tile_pool(name="xsb", bufs=1))
        xsb = []
        for dt_ in range(DT):
            xt_ = xsb_pool.tile([P, S], BF16, tag=f"x{dt_}", name=f"x{dt_}")
            nc.sync.dma_start(out=xt_, in_=xT[dt_ * P:(dt_ + 1) * P, :])
            xsb.append(xt_)
        load_w("v", wv)
        load_w("q", wq)

        # normalized ctx per group (own token half only), persists until Wo
        ctxn_pool = es_fw.enter_context(tc.tile_pool(name="ctxn", bufs=1))
        ctxn = [ctxn_pool.tile([P, T], BF16, tag=f"cn{g}", name=f"cn{g}")
                for g in range(HG)]

        # AllGather bounce buffers (HBM)
        agin = [dram.tile([P, T], BF16, tag=f"agi{g}", name=f"agi{g}")
                for g in range(HG)]
        agout = [dram.tile([2, P, T], BF16, tag=f"ago{g}", name=f"ago{g}")
                 for g in range(HG)]

        # =============== Phase F: fused QKV + attention per group ========
        with tc.tile_pool(name="kbuf", bufs=2) as kbuf_pool, \
             tc.tile_pool(name="qbuf", bufs=2) as qbuf_pool, \
             tc.tile_pool(name="vbuf", bufs=2) as vbuf_pool, \
             tc.tile_pool(name="vtmp", bufs=2) as vtmp_pool, \
             tc.tile_pool(name="ctx", bufs=2) as ctx_pool, \
             tc.tile_pool(name="pbuf", bufs=4) as pbuf_pool, \
             tc.tile_pool(name="rcp", bufs=2) as rcp_pool, \
             tc.tile_pool(name="psS", bufs=2, space="PSUM") as psS, \
             tc.tile_pool(name="psC", bufs=2, space="PSUM") as psC, \
             tc.tile_pool(name="psA", bufs=1, space="PSUM") as psA, \
             tc.tile_pool(name="psX", bufs=1, space="PSUM") as psX:

            for g in range(HG):
                # ---- projections for group g (2 heads = 128 features) ----
                ksb = kbuf_pool.tile([P, S], BF16, tag="kb", name="kb")
                # Q staged zero-padded per head: sibling head rows zero so
                # the full [128,128] K stationary tiles cancel them
                qz = [qbuf_pool.tile([P, S], BF16, tag=f"qz{hh}",
                                     name=f"qz{hh}") for hh in range(2)]
                for hh in range(2):
                    nc.vector.memset(qz[hh][:], 0.0)
                vsb = vbuf_pool.tile([P, KT, 2, HD + 1], BF16, tag="vb",
                                     name="vb")

                def proj(wtiles, bias, dest_cb, g=g):
                    for ch in range(S // 512):
                        ps = psA.tile([P, 512], F32, tag="psA", name="psA")
                        for din in range(DT):
                            nc.tensor.matmul(
                                ps[:], wtiles[din][:, g, :],
                                xsb[din][:, ch * 512:(ch + 1) * 512],
                                start=(din == 0), stop=(din == DT - 1))
                        dest_cb(ps, ch)

                def k_evict(ps, ch, ksb=ksb, g=g):
                    nc.vector.tensor_scalar(
                        out=ksb[:, ch * 512:(ch + 1) * 512], in0=ps[:],
                        scalar1=biases["bk_p"][:, g:g + 1], scalar2=None,
                        op0=OP.add)

                def q_evict(ps, ch, qz=qz, g=g):
                    for hh in range(2):
                        r0 = hh * HD
                        nc.vector.tensor_scalar(
                            out=qz[hh][r0:r0 + HD, ch * 512:(ch + 1) * 512],
                            in0=ps[r0:r0 + HD, :],
                            scalar1=biases["bq_p"][r0:r0 + HD, g:g + 1],
                            scalar2=None, op0=OP.add)

                def v_evict(ps, ch, vsb=vsb, g=g):
                    vt = vtmp_pool.tile([P, 512], BF16, tag="vt", name="vt")
                    nc.vector.tensor_scalar(
                        out=vt[:], in0=ps[:],
                        scalar1=biases["bv_p"][:, g:g + 1], scalar2=None,
                        op0=OP.add)
                    for hh in range(2):
                        idsl = ident[hh * HD:(hh + 1) * HD,
                                     hh * HD:(hh + 1) * HD]
                        for st in range(4):
                            pt = psX.tile([P, HD], BF16, tag="psX",
                                          name="psX")
                            nc.tensor.transpose(
                                pt[:],
                                vt[hh * HD:(hh + 1) * HD,
                                   st * P:(st + 1) * P], idsl)
                            nc.vector.tensor_copy(
                                vsb[:, ch * 4 + st, hh, 0:HD], pt[:])

                proj(wsl["k"], "bk_p", k_evict)
                proj(wsl["v"], "bv_p", v_evict)
                nc.vector.memset(vsb[:, :, :, HD:HD + 1], 1.0)
                proj(wsl["q"], "bq_p", q_evict)

                # ---- attention for the 2 heads over all 2048 queries ----
                # pass 0: own token half -> ctxn[g]; pass 1: peer -> send
                send = ctx_pool.tile([P, T], BF16, tag="send", name="send")
                for ps_ in (1, 0):       # peer tokens first -> AG launches
                    p0 = ps_ * 1024      # earlier and overlaps more compute
                    for hh in range(2):
                        r0 = hh * HD
                        cpss = [psC.tile([HD + 1, 512], F32, tag="cps",
                                         name="cps") for _ in range(2)]
                        for jp in range(KT // 2):
                            j0, j1 = 2 * jp, 2 * jp + 1
                            for qc in range(2):
                                c0 = p0 + qc * 512
                                sps = psS.tile([P, 2, 512], F32, tag="sps",
                                               name="sps")
                                nc.tensor.matmul(
                                    sps[:, 0, :],
                                    ksb[:, j0 * P:(j0 + 1) * P],
                                    qz[hh][:, c0:c0 + 512],
                                    start=True, stop=True)
                                nc.tensor.matmul(
                                    sps[:, 1, :],
                                    ksb[:, j1 * P:(j1 + 1) * P],
                                    qz[hh][:, c0:c0 + 512],
                                    start=True, stop=True)
                                pT = pbuf_pool.tile([P, 2, 512], BF16,
                                                    tag="pT", name="pT")
                                nc.scalar.activation(pT[:], sps[:], AF.Exp)
                                nc.tensor.matmul(
                                    cpss[qc][:], vsb[:, j0, hh, :],
                                    pT[:, 0, :],
                                    start=(jp == 0), stop=False)
                                nc.tensor.matmul(
                                    cpss[qc][:], vsb[:, j1, hh, :],
                                    pT[:, 1, :],
                                    start=False, stop=(jp == KT // 2 - 1))
                        # evict + normalize by softmax denominator (row 64)
                        dest = ctxn[g] if ps_ == 0 else send
                        for qc in range(2):
                            ctx_sb = ctx_pool.tile([HD + 1, 512], BF16,
                                                   tag="cs", name="cs")
                            nc.vector.tensor_copy(ctx_sb[:], cpss[qc][:])
                            # same-size buffer as cps: rotate in psC so the
                            # den matmul never blocks psA's projection chains
                            dn = psC.tile([HD + 1, 512], F32, tag="cps",
                                          name="dn")
                            nc.tensor.matmul(
                                dn[0:HD, :], selD[:, :], ctx_sb[:],
                                start=True, stop=True)
                            rc = rcp_pool.tile([HD, 512], F32, tag="rc",
                                               name="rc")
                            nc.vector.reciprocal(rc[:], dn[0:HD, :])
                            nc.vector.tensor_tensor(
                                out=dest[r0:r0 + HD,
                                         qc * 512:(qc + 1) * 512],
                                in0=ctx_sb[0:HD, :],
                                in1=rc[:], op=OP.mult)

                # ---- ship peer's token half to the pair core ------------
                nc.gpsimd.dma_start(agin[g][:, :], send[:])
                nc.gpsimd.collective_compute(
                    "AllGather", OP.bypass, replica_groups=GROUPS,
                    ins=[agin[g][:, :].opt()],
                    outs=[agout[g][:, :, :].opt()])

            # Fence: a 5th tiny AllGather.  The tile framework makes each
            # collective's input writer wait until all PRIOR collectives'
            # data has ARRIVED (the trigger instruction itself completes
            # early), so the fence writer + gpsimd program order make the
            # agout readbacks below race-free.
            fence_sb = const.tile([P, 8], BF16, tag="fsb", name="fsb")
            nc.vector.memset(fence_sb[:], 0.0)
            fence_in = dram.tile([P, 8], BF16, tag="fin", name="fin")
            fence_out = dram.tile([2, P, 8], BF16, tag="fout", name="fout")
            nc.gpsimd.dma_start(fence_in[:, :], fence_sb[:])
            nc.gpsimd.collective_compute(
                "AllGather", OP.bypass, replica_groups=GROUPS,
                ins=[fence_in[:, :].opt()],
                outs=[fence_out[:, :, :].opt()])

        # =============== Phase W: Wo + residual + LN1 ===================
        with tc.tile_pool(name="asb", bufs=1) as asb_pool, \
             tc.tile_pool(name="zT", bufs=1) as zT_pool, \
             tc.tile_pool(name="ln1", bufs=2) as ln1_pool, \
             tc.tile_pool(name="psW", bufs=3, space="PSUM") as psW, \
             tc.tile_pool(name="psStat", bufs=1, space="PSUM") as psStat, \
             tc.tile_pool(name="psBc", bufs=1, space="PSUM") as psBc:
            wo_sb = [wx_pool.tile([P, D], BF16, tag="wx", name="wx")
                     for _ in range(12)]
            for i in range(12):
                nc.sync.dma_start(out=wo_sb[i], in_=wo[i * P:(i + 1) * P, :])
            # FFN hb=0 W1 preload (overlaps W phase)
            w1b0 = [w1p_pool.tile([P, D], BF16, tag="w1p", name="w1p")
                    for _ in range(DT)]
            for i in range(DT):
                nc.sync.dma_start(out=w1b0[i], in_=w1[i * P:(i + 1) * P,
                                                      0:1024])

            # peer ctx contributions from the AllGathers (both slots; the
            # useless slot's wo rows are zero)
            asb = []
            for g in range(HG):
                for s in range(2):
                    a = asb_pool.tile([P, T], BF16, tag=f"a{g}{s}",
                                      name=f"a{g}{s}")
                    # gpsimd queue: executes after the fence writer above,
                    # i.e. after every AllGather's data has arrived
                    nc.gpsimd.dma_start(a[:], agout[g][s, :, :])
                    asb.append(a)
            # moving operands for Wo in wo_sb row order:
            #   rows 0:512   -> own ctxn groups 0..3 (own tokens = cols 0:T)
            #   rows 512:1536 -> agout g0 s0, g0 s1, g1 s0, ... (g3 last so
            #   the last AllGather's wait overlaps the first 10 matmuls)
            movs = [ctxn[g][:] for g in range(HG)] + asb

            zT = [zT_pool.tile([P, T], BF16, tag=f"zT{i}", name=f"zT{i}")
                  for i in range(DT)]
            for ch in range(T // 512):
                for dout in range(DT):
                    ps = psW.tile([P, 512], F32, tag="psW", name="psW")
                    for din in range(12):
                        nc.tensor.matmul(
                            ps[:], wo_sb[din][:, dout * P:(dout + 1) * P],
                            movs[din][:, ch * 512:(ch + 1) * 512],
                            start=(din == 0), stop=(din == 11))
                    # z = attn_out + bo + x_resid
                    nc.vector.scalar_tensor_tensor(
                        zT[dout][:, ch * 512:(ch + 1) * 512], ps[:],
                        biases["bo_p"][:, dout:dout + 1],
                        xsb[dout][:, ch * 512:(ch + 1) * 512],
                        op0=OP.add, op1=OP.add)

            # ---- LN1 (feature-major; stats over partitions via PE) -----
            for ch in range(T // 512):
                sl = slice(ch * 512, (ch + 1) * 512)
                sum_ps = psStat.tile([1, 512], F32, tag="s", name="s")
                sq_ps = psStat.tile([1, 512], F32, tag="q", name="q")
                for dt_ in range(DT):
                    zsq = ln1_pool.tile([P, 512], BF16, tag="zsq",
                                        name="zsq")
                    nc.scalar.activation(zsq[:], zT[dt_][:, sl], AF.Square)
                    nc.tensor.matmul(sum_ps[:], ones_col[:], zT[dt_][:, sl],
                                     start=(dt_ == 0), stop=(dt_ == DT - 1))
                    nc.tensor.matmul(sq_ps[:], ones_col[:], zsq[:],
                                     start=(dt_ == 0), stop=(dt_ == DT - 1))
                mean = ln1_pool.tile([1, 512], F32, tag="mean", name="mean")
                nc.scalar.mul(mean[:], sum_ps[:], 1.0 / D)
                msq = ln1_pool.tile([1, 512], F32, tag="msq", name="msq")
                nc.scalar.mul(msq[:], sq_ps[:], 1.0 / D)
                m2 = ln1_pool.tile([1, 512], F32, tag="m2", name="m2")
                nc.vector.tensor_mul(m2[:], mean[:], mean[:])
                var = ln1_pool.tile([1, 512], F32, tag="var", name="var")
                nc.vector.tensor_sub(var[:], msq[:], m2[:])
                std = ln1_pool.tile([1, 512], F32, tag="std", name="std")
                nc.scalar.activation(std[:], var[:], AF.Sqrt,
                                     bias=eps_sb[0:1, :])
                rstd = ln1_pool.tile([1, 512], F32, tag="rstd", name="rstd")
                nc.vector.reciprocal(rstd[:], std[:])
                mean_r = ln1_pool.tile([1, 512], BF16, tag="meanr",
                                       name="meanr")
                nc.vector.tensor_copy(mean_r[:], mean[:])
                rstd_r = ln1_pool.tile([1, 512], BF16, tag="rstdr",
                                       name="rstdr")
                nc.vector.tensor_copy(rstd_r[:], rstd[:])
                bm_ps = psBc.tile([P, 512], F32, tag="bm", name="bm")
                nc.tensor.matmul(bm_ps[:], ones_row[:], mean_r[:],
                                 start=True, stop=True)
                br_ps = psBc.tile([P, 512], F32, tag="br", name="br")
                nc.tensor.matmul(br_ps[:], ones_row[:], rstd_r[:],
                                 start=True, stop=True)
                bm = ln1_pool.tile([P, 512], F32, tag="bm_sb", name="bm_sb")
                nc.scalar.copy(bm[:], bm_ps[:])
                br = ln1_pool.tile([P, 512], F32, tag="br_sb", name="br_sb")
                nc.scalar.copy(br[:], br_ps[:])
                for dt_ in range(DT):
                    tmp = ln1_pool.tile([P, 512], F32, tag="n1", name="n1")
                    nc.vector.scalar_tensor_tensor(
                        tmp[:], zT[dt_][:, sl],
                        1.0, bm[:], op0=OP.mult, op1=OP.subtract)
                    tmp2 = ln1_pool.tile([P, 512], F32, tag="n2", name="n2")
                    nc.vector.scalar_tensor_tensor(
                        tmp2[:], tmp[:],
                        biases["g1_p"][:, dt_:dt_ + 1], br[:],
                        op0=OP.mult, op1=OP.mult)
                    nc.scalar.activation(
                        hT[dt_][:, sl], tmp2[:], AF.Identity,
                        bias=biases["be1_p"][:, dt_:dt_ + 1])

        es_fw.close()   # free x / QKV weights / ctx SBUF before the FFN

        # =============== Phase 4: FFN + residual + fused LN2/out ========
        with tc.tile_pool(name="wxf", bufs=20) as wxf_pool, \
             tc.tile_pool(name="z2T", bufs=1) as z2T_pool, \
             tc.tile_pool(name="t1", bufs=12) as t1_pool, \
             tc.tile_pool(name="ztmp", bufs=2) as ztmp_pool, \
             tc.tile_pool(name="o2", bufs=1) as o2_pool, \
             tc.tile_pool(name="tm", bufs=2) as tm_pool, \
             tc.tile_pool(name="lnc", bufs=1) as lnc_pool, \
             tc.tile_pool(name="ln2", bufs=2) as ln2_pool, \
             tc.tile_pool(name="psF1", bufs=3, space="PSUM") as psF1, \
             tc.tile_pool(name="psF2", bufs=3, space="PSUM") as psF2, \
             tc.tile_pool(name="psT5", bufs=2, space="PSUM") as psT5:
            z2T = [z2T_pool.tile([P, T], BF16, tag=f"z2T{i}", name=f"z2T{i}")
                   for i in range(DT)]
            out2 = [o2_pool.tile([P, T], F32, tag=f"o2{i}", name=f"o2{i}")
                    for i in range(DT)]
            g2_bc = lnc_pool.tile([P, D], F32, tag="g2bc", name="g2bc")
            nc.sync.dma_start(out=g2_bc, in_=bass.AP(
                tensor=t["g2_d"], offset=0, ap=[[0, P], [1, D]]))
            be2_bc = lnc_pool.tile([P, D], F32, tag="be2bc", name="be2bc")
            nc.sync.dma_start(out=be2_bc, in_=bass.AP(
                tensor=t["be2_d"], offset=0, ap=[[0, P], [1, D]]))

            def phase5(nts):
                for nt in nts:
                    z2 = tm_pool.tile([P, D], F32, tag="z2tm", name="z2tm")
                    for dt_ in range(DT):
                        pt = psT5.tile([P, P], BF16, tag="psT5", name="psT5")
                        nc.tensor.transpose(
                            pt[:], z2T[dt_][:, nt * P:(nt + 1) * P],
                            ident[:])
                        nc.scalar.copy(z2[:, dt_ * P:(dt_ + 1) * P], pt[:])
                    stats = ln2_pool.tile([P, 2, 6], F32, tag="st", name="st")
                    for gg in range(2):
                        nc.vector.bn_stats(out=stats[:, gg, :],
                                           in_=z2[:, gg * 512:(gg + 1) * 512])
                    mv = ln2_pool.tile([P, 2], F32, tag="mv", name="mv")
                    nc.vector.bn_aggr(out=mv[:], in_=stats[:])
                    std = ln2_pool.tile([P, 1], F32, tag="std2", name="std2")
                    nc.scalar.activation(std[:], mv[:, 1:2], AF.Sqrt,
                                         bias=eps_sb[:])
                    rstd = ln2_pool.tile([P, 1], F32, tag="rstd2",
                                         name="rstd2")
                    nc.vector.reciprocal(rstd[:], std[:])
                    xn = ln2_pool.tile([P, D], F32, tag="xn", name="xn")
                    nc.vector.tensor_scalar(
                        out=xn[:], in0=z2[:], scalar1=mv[:, 0:1],
                        scalar2=rstd[:], op0=OP.subtract, op1=OP.mult)
                    xg = ln2_pool.tile([P, D], F32, tag="xg", name="xg")
                    nc.vector.tensor_mul(xg[:], xn[:], g2_bc[:])
                    fin = ln2_pool.tile([P, D], F32, tag="fin", name="fin")
                    nc.vector.tensor_add(fin[:], xg[:], be2_bc[:])
                    nc.sync.dma_start(out=out[nt * P:(nt + 1) * P, :],
                                      in_=fin[:])

            for hb in range(4):              # hidden blocks of 1024
                if hb == 0:
                    w1b = w1b0
                else:
                    w1b = [wxf_pool.tile([P, D], BF16, tag="wxf", name="wxf")
                           for _ in range(DT)]
                    for i in range(DT):
                        nc.sync.dma_start(
                            out=w1b[i],
                            in_=w1[i * P:(i + 1) * P,
                                   hb * 1024:(hb + 1) * 1024])
                w2b = [wxf_pool.tile([P, D], BF16, tag="wxf", name="wxf")
                       for _ in range(DT)]
                for i in range(DT):
                    nc.sync.dma_start(
                        out=w2b[i],
                        in_=w2[(hb * 8 + i) * P:(hb * 8 + i + 1) * P, :])
                for tc4 in range(T // 512):  # 2 token chunks of 512
                    tsl = slice(tc4 * 512, (tc4 + 1) * 512)
                    t1s = []
                    for i in range(DT):      # 8 hidden tiles in block
                        t1ps = psF1.tile([P, 512], F32, tag="t1ps",
                                         name="t1ps")
                        for din in range(DT):
                            nc.tensor.matmul(
                                t1ps[:], w1b[din][:, i * P:(i + 1) * P],
                                hT[din][:, tsl],
                                start=(din == 0), stop=(din == DT - 1))
                        t1 = t1_pool.tile([P, 512], BF16, tag="t1", name="t1")
                        nc.scalar.activation(
                            t1[:], t1ps[:], AF.Relu,
                            bias=b1_sb[:, hb * 8 + i:hb * 8 + i + 1])
                        t1s.append(t1)
                    for dout in range(DT):
                        o2ps = psF2.tile([P, 512], F32, tag="o2ps",
                                         name="o2ps")
                        for i in range(DT):
                            nc.tensor.matmul(
                                o2ps[:], w2b[i][:, dout * P:(dout + 1) * P],
                                t1s[i][:],
                                start=(i == 0), stop=(i == DT - 1))
                        if hb == 0:
                            nc.vector.tensor_copy(out2[dout][:, tsl],
                                                  o2ps[:])
                        elif hb < 3:
                            nc.vector.tensor_tensor(
                                out=out2[dout][:, tsl], in0=o2ps[:],
                                in1=out2[dout][:, tsl], op=OP.add)
                        else:
                            # last block: fold in residual + b2 -> z2T
                            tmp = ztmp_pool.tile([P, 512], F32, tag="zt",
                                                 name="zt")
                            nc.vector.tensor_tensor(
                                out=tmp[:], in0=o2ps[:],
                                in1=out2[dout][:, tsl], op=OP.add)
                            nc.vector.scalar_tensor_tensor(
                                z2T[dout][:, tsl], tmp[:],
                                biases["b2_p"][:, dout:dout + 1],
                                hT[dout][:, tsl], op0=OP.add, op1=OP.add)
                    if hb == 3:
                        # LN2 + output for this half while the other half
                        # of the FFN (or nothing) still runs
                        phase5(range(tc4 * 4, (tc4 + 1) * 4))


def _selD():
    sel = np.zeros((HD + 1, HD), dtype=np.float32)
    sel[HD, :] = 1.0
    return sel


def _pack(v, nt):
    return np.ascontiguousarray(v.reshape(nt, P).T)


def kernel(x, Wq, bq, Wk, bk, Wv, bv, Wo, bo, W1, b1, W2, b2, g1, beta1,
           g2, beta2):
    x = np.asarray(x, dtype=np.float32)
    if "nc" not in _CACHED:
        _CACHED["nc"] = _build_program()
    nc = _CACHED["nc"]

    import ml_dtypes
    bf16 = lambda a: np.ascontiguousarray(
        np.asarray(a, dtype=np.float32).astype(ml_dtypes.bfloat16))
    f32 = lambda a: np.ascontiguousarray(np.asarray(a, dtype=np.float32))
    scale = 1.0 / np.sqrt(HD)
    Wk_s = np.asarray(Wk, np.float64) * scale
    bk_s = f32(bk) * scale

    common = {
        "w1": bf16(W1), "w2": bf16(W2),
        "bo_p": _pack(f32(bo), DT), "b1_p": _pack(f32(b1), FT),
        "b2_p": _pack(f32(b2), DT),
        "g1_p": _pack(f32(g1), DT), "be1_p": _pack(f32(beta1), DT),
        "g2_d": f32(g2), "be2_d": f32(beta2),
        "ident_d": np.eye(P).astype(ml_dtypes.bfloat16),
        "ones_row_d": np.ones((1, P)).astype(ml_dtypes.bfloat16),
        "ones_col_d": np.ones((P, 1)).astype(ml_dtypes.bfloat16),
        "selD_d": _selD().astype(ml_dtypes.bfloat16),
    }
    in_maps = []
    for c in range(N_CORES):
        b, half = c // 2, c % 2
        own = x[b, half * T:(half + 1) * T]           # [1024, 1024]
        other = x[b, (1 - half) * T:(2 - half) * T]
        xT_c = np.ascontiguousarray(
            np.concatenate([own, other], axis=0).T).astype(
                ml_dtypes.bfloat16)                   # [1024, 2048]
        hsl = slice(half * 512, (half + 1) * 512)
        # [D, 512] -> [D, 4, 128] (group-major, natural order)
        wq_c = bf16(np.asarray(Wq)[:, hsl]).reshape(D, HG, P)
        wk_c = bf16(Wk_s[:, hsl]).reshape(D, HG, P)
        wv_c = bf16(np.asarray(Wv)[:, hsl]).reshape(D, HG, P)
        # wo rows: own 512 (natural), then per group g: slot0, slot1 where
        # slot s holds Wo rows of heads (s*8 + 2g, s*8+2g+1) if s != half
        # else zeros (that slot of the AllGather is this core's own data
        # for the peer's tokens -- not used here).
        Wo_np = np.asarray(Wo, np.float32)
        rows = [Wo_np[hsl]]
        for g in range(HG):
            for s in range(2):
                if s != half:
                    rows.append(Wo_np[s * 512 + g * P: s * 512 + (g + 1) * P])
                else:
                    rows.append(np.zeros((P, D), np.float32))
        wo_c = bf16(np.concatenate(rows, axis=0))     # [1536, 1024]
        bqp = _pack(f32(bq)[hsl], HG)
        bkp = _pack(bk_s[hsl], HG)
        bvp = _pack(f32(bv)[hsl], HG)
        in_maps.append({**common, "xT": np.ascontiguousarray(xT_c),
                        "wq": np.ascontiguousarray(wq_c),
                        "wk": np.ascontiguousarray(wk_c),
                        "wv": np.ascontiguousarray(wv_c),
                        "wo": wo_c,
                        "bq_p": bqp, "bk_p": bkp, "bv_p": bvp})

    trace = bool(os.environ.get("KERNEL_TRACE"))
    res = bass_utils.run_bass_kernel_spmd(
        nc, in_maps, core_ids=list(range(N_CORES)), trace=trace)
    _CACHED["last_result"] = res

    y = np.empty((4, S, D), dtype=np.float32)
    for c in range(N_CORES):
        b, half = c // 2, c % 2
        y[b, half * T:(half + 1) * T] = res.results[c]["out"]
    return y


# revision 34
# speedup vs baseline: 1.2199x; 1.0040x over previous
"""Trainium2 Bass kernel for a dense transformer layer (attention + FFN + 2 LayerNorms).

Problem shapes: x [4, 2048, 1024], d_model=1024, heads=16 (hd=64), d_ff=4096.

Sharding (8 cores): core c handles batch b = c//2.  The core PAIR (2b, 2b+1)
splits the layer two ways:
  - attention is HEAD-sharded: core half=c%2 computes Q/K/V and attention for
    its 8 heads over ALL 2048 tokens of the batch (no duplicated K/V work);
  - everything after attention (Wo, LN1, FFN, LN2) is TOKEN-sharded: each
    core owns 1024 tokens (host permutes x so own tokens are columns 0:1023).
A tiny pairwise AllGather per 2-head group moves the normalized attention
context for the peer's token half across the pair (4 x 256KB, overlapped with
the remaining attention groups).  The peer's contribution enters Wo as extra
stationary row-blocks whose unused slot rows are zeroed host-side, so no
per-core control flow is needed (pure SPMD).

Layout: activations feature-major (features on partitions, tokens free).
Scores are computed per head with contraction 64 (S^T = K_h Q_h^T, keys on
partitions), exp on the scalar engine over [128,1024] PSUM tiles, ctx via
V_aug (ones column -> softmax denominator for free).  QKV projection of group
g+1 overlaps the scalar-bound attention of group g on the PE.
"""

import os
import numpy as np

import concourse.bass as bass
import concourse.tile as tile
from concourse import bacc, mybir
from concourse import bass_utils

BF16 = mybir.dt.bfloat16
F32 = mybir.dt.float32
AF = mybir.ActivationFunctionType
OP = mybir.AluOpType

D = 1024          # d_model
S = 2048          # full sequence per batch
T = 1024          # tokens owned per core (post-attention)
H = 16            # total heads
HG = 4            # local 2-head groups per core (8 heads)
HD = 64           # head dim
F = 4096          # ffn hidden
P = 128
DT = D // P       # 8 feature tiles
KT = S // P       # 16 key-token tiles
FT = F // P       # 32 hidden tiles
N_CORES = 8
EPS = 1e-5
GROUPS = [[0, 1], [2, 3], [4, 5], [6, 7]]

_CACHED = {}


def _build_program():
    nc = bacc.Bacc("TRN2", target_bir_lowering=False, debug=False,
                   num_devices=N_CORES)

    tens = {}

    def di(name, shape, dtype=BF16):
        tens[name] = nc.dram_tensor(name, shape, dtype, kind="ExternalInput")

    di("xT", [D, S])
    di("wq", [D, HG, P]); di("wk", [D, HG, P]); di("wv", [D, HG, P])
    di("wo", [12 * P, D])
    di("w1", [D, F]); di("w2", [F, D])
    for nm in ["bq_p", "bk_p", "bv_p"]:
        di(nm, [P, HG], F32)
    for nm in ["bo_p", "b2_p", "g1_p", "be1_p"]:
        di(nm, [P, DT], F32)
    di("b1_p", [P, FT], F32)
    di("g2_d", [D], F32); di("be2_d", [D], F32)
    di("ident_d", [P, P]); di("ones_row_d", [1, P]); di("ones_col_d", [P, 1])
    di("selD_d", [HD + 1, HD])
    tens["out"] = nc.dram_tensor("out", [T, D], F32, kind="ExternalOutput")

    with tile.TileContext(nc) as tc:
        _trace_kernel(nc, tc, tens)
    nc.compile()
    return nc


def _trace_kernel(nc, tc, t):
    xT, wq, wk, wv, wo, w1, w2 = (t["xT"], t["wq"], t["wk"], t["wv"], t["wo"],
                                  t["w1"], t["w2"])
    out = t["out"]

    from contextlib import ExitStack
    es = ExitStack()
    with es:
        dram = es.enter_context(tc.tile_pool(name="dram", bufs=1, space="DRAM"))

        const = es.enter_context(tc.tile_pool(name="const", bufs=1))
        ident = const.tile([P, P], BF16, tag="ident", name="ident")
        nc.sync.dma_start(out=ident, in_=t["ident_d"][:, :])
        ones_row = const.tile([1, P], BF16, tag="onesr", name="onesr")
        nc.sync.dma_start(out=ones_row, in_=t["ones_row_d"][:, :])
        ones_col = const.tile([P, 1], BF16, tag="onesc", name="onesc")
        nc.sync.dma_start(out=ones_col, in_=t["ones_col_d"][:, :])
        selD = const.tile([HD + 1, HD], BF16, tag="selD", name="selD")
        nc.sync.dma_start(out=selD, in_=t["selD_d"][:, :])
        biases = {}
        for name in ["bq_p", "bk_p", "bv_p"]:
            bt = const.tile([P, HG], F32, tag=name)
            nc.sync.dma_start(out=bt, in_=t[name][:, :])
            biases[name] = bt
        for name in ["bo_p", "b2_p", "g1_p", "be1_p"]:
            bt = const.tile([P, DT], F32, tag=name)
            nc.sync.dma_start(out=bt, in_=t[name][:, :])
            biases[name] = bt
        eps_sb = const.tile([P, 1], F32, tag="eps", name="eps")
        nc.vector.memset(eps_sb[:], EPS)
        b1_sb = const.tile([P, FT], F32, tag="b1", name="b1")
        nc.sync.dma_start(out=b1_sb, in_=t["b1_p"][:, :])

        # long-lived pools (outlive phase W into the FFN) must open before
        # the phase-F/W pools below them on the allocation stack
        hT_pool = es.enter_context(tc.tile_pool(name="hT", bufs=1))
        hT = [hT_pool.tile([P, T], BF16, tag=f"hT{i}", name=f"hT{i}")
              for i in range(DT)]
        wx_pool = es.enter_context(tc.tile_pool(name="wx", bufs=12))
        w1p_pool = es.enter_context(tc.tile_pool(name="w1p", bufs=8))

        # pools that die after phase W (freed before the FFN needs SBUF)
        es_fw = ExitStack()
        # K weights first (small) so the first projection chain isn't
        # queued behind the 4MB x load, then x, then Q/V weights
        wqkv_pool = es_fw.enter_context(tc.tile_pool(name="wqkv", bufs=1))
        wsl = {}

        def load_w(nm, wd):
            tiles = []
            for din in range(DT):
                wt = wqkv_pool.tile([P, HG, P], BF16, tag=f"w{nm}{din}",
                                    name=f"w{nm}{din}")
                nc.sync.dma_start(out=wt, in_=wd[din * P:(din + 1) * P, :, :])
                tiles.append(wt)
            wsl[nm] = tiles

        load_w("k", wk)
        xsb_pool = es_fw.enter_context(tc.tile_pool(name="xsb", bufs=1))
        xsb = []
        for dt_ in range(DT):
            xt_ = xsb_pool.tile([P, S], BF16, tag=f"x{dt_}", name=f"x{dt_}")
            nc.sync.dma_start(out=xt_, in_=xT[dt_ * P:(dt_ + 1) * P, :])
            xsb.append(xt_)
        load_w("v", wv)
        load_w("q", wq)

        # normalized ctx per group (own token half only), persists until Wo
        ctxn_pool = es_fw.enter_context(tc.tile_pool(name="ctxn", bufs=1))
        ctxn = [ctxn_pool.tile([P, T], BF16, tag=f"cn{g}", name=f"cn{g}")
                for g in range(HG)]

        # AllGather bounce buffers (HBM)
        agin = [dram.tile([P, T], BF16, tag=f"agi{g}", name=f"agi{g}")
                for g in range(HG)]
        agout = [dram.tile([2, P, T], BF16, tag=f"ago{g}", name=f"ago{g}")
                 for g in range(HG)]

        # =============== Phase F: fused QKV + attention per group ========
        with tc.tile_pool(name="kbuf", bufs=2) as kbuf_pool, \
             tc.tile_pool(name="qbuf", bufs=2) as qbuf_pool, \
             tc.tile_pool(name="vbuf", bufs=2) as vbuf_pool, \
             tc.tile_pool(name="vtmp", bufs=2) as vtmp_pool, \
             tc.tile_pool(name="ctx", bufs=2) as ctx_pool, \
             tc.tile_pool(name="pbuf", bufs=4) as pbuf_pool, \
             tc.tile_pool(name="rcp", bufs=2) as rcp_pool, \
             tc.tile_pool(name="psS", bufs=2, space="PSUM") as psS, \
             tc.tile_pool(name="psC", bufs=2, space="PSUM") as psC, \
             tc.tile_pool(name="psA", bufs=1, space="PSUM") as psA, \
             tc.tile_pool(name="psX", bufs=1, space="PSUM") as psX:

            for g in range(HG):
                # ---- projections for group g (2 heads = 128 features) ----
                ksb = kbuf_pool.tile([P, S], BF16, tag="kb", name="kb")
                # Q staged zero-padded per head: sibling head rows zero so
                # the full [128,128] K stationary tiles cancel them
                qz = [qbuf_pool.tile([P, S], BF16, tag=f"qz{hh}",
                                     name=f"qz{hh}") for hh in range(2)]
                for hh in range(2):
                    nc.vector.memset(qz[hh][:], 0.0)
                vsb = vbuf_pool.tile([P, KT, 2, HD + 1], BF16, tag="vb",
                                     name="vb")

                def proj(wtiles, bias, dest_cb, g=g):
                    for ch in range(S // 512):
                        ps = psA.tile([P, 512], F32, tag="psA", name="psA")
                        for din in range(DT):
                            nc.tensor.matmul(
                                ps[:], wtiles[din][:, g, :],
                                xsb[din][:, ch * 512:(ch + 1) * 512],
                                start=(din == 0), stop=(din == DT - 1))
                        dest_cb(ps, ch)

                def k_evict(ps, ch, ksb=ksb, g=g):
                    nc.vector.tensor_scalar(
                        out=ksb[:, ch * 512:(ch + 1) * 512], in0=ps[:],
                        scalar1=biases["bk_p"][:, g:g + 1], scalar2=None,
                        op0=OP.add)

                def q_evict(ps, ch, qz=qz, g=g):
                    for hh in range(2):
                        r0 = hh * HD
                        nc.vector.tensor_scalar(
                            out=qz[hh][r0:r0 + HD, ch * 512:(ch + 1) * 512],
                            in0=ps[r0:r0 + HD, :],
                            scalar1=biases["bq_p"][r0:r0 + HD, g:g + 1],
                            scalar2=None, op0=OP.add)

                def v_evict(ps, ch, vsb=vsb, g=g):
                    vt = vtmp_pool.tile([P, 512], BF16, tag="vt", name="vt")
                    nc.vector.tensor_scalar(
                        out=vt[:], in0=ps[:],
                        scalar1=biases["bv_p"][:, g:g + 1], scalar2=None,
                        op0=OP.add)
                    for hh in range(2):
                        idsl = ident[hh * HD:(hh + 1) * HD,
                                     hh * HD:(hh + 1) * HD]
                        for st in range(4):
                            pt = psX.tile([P, HD], BF16, tag="psX",
                                          name="psX")
                            nc.tensor.transpose(
                                pt[:],
                                vt[hh * HD:(hh + 1) * HD,
                                   st * P:(st + 1) * P], idsl)
                            nc.vector.tensor_copy(
                                vsb[:, ch * 4 + st, hh, 0:HD], pt[:])

                proj(wsl["k"], "bk_p", k_evict)
                proj(wsl["v"], "bv_p", v_evict)
                nc.vector.memset(vsb[:, :, :, HD:HD + 1], 1.0)
                proj(wsl["q"], "bq_p", q_evict)

                # ---- attention for the 2 heads over all 2048 queries ----
                # pass 0: own token half -> ctxn[g]; pass 1: peer -> send
                send = ctx_pool.tile([P, T], BF16, tag="send", name="send")
                for ps_ in (1, 0):       # peer tokens first -> AG launches
                    p0 = ps_ * 1024      # earlier and overlaps more compute
                    for hh in range(2):
                        r0 = hh * HD
                        cpss = [psC.tile([HD + 1, 512], F32, tag="cps",
                                         name="cps") for _ in range(2)]
                        for jp in range(KT // 2):
                            j0, j1 = 2 * jp, 2 * jp + 1
                            for qc in range(2):
                                c0 = p0 + qc * 512
                                sps = psS.tile([P, 2, 512], F32, tag="sps",
                                               name="sps")
                                nc.tensor.matmul(
                                    sps[:, 0, :],
                                    ksb[:, j0 * P:(j0 + 1) * P],
                                    qz[hh][:, c0:c0 + 512],
                                    start=True, stop=True)
                                nc.tensor.matmul(
                                    sps[:, 1, :],
                                    ksb[:, j1 * P:(j1 + 1) * P],
                                    qz[hh][:, c0:c0 + 512],
                                    start=True, stop=True)
                                pT = pbuf_pool.tile([P, 2, 512], BF16,
                                                    tag="pT", name="pT")
                                nc.scalar.activation(pT[:], sps[:], AF.Exp)
                                nc.tensor.matmul(
                                    cpss[qc][:], vsb[:, j0, hh, :],
                                    pT[:, 0, :],
                                    start=(jp == 0), stop=False)
                                nc.tensor.matmul(
                                    cpss[qc][:], vsb[:, j1, hh, :],
                                    pT[:, 1, :],
                                    start=False, stop=(jp == KT // 2 - 1))
                        # evict + normalize by softmax denominator (row 64)
                        dest = ctxn[g] if ps_ == 0 else send
                        for qc in range(2):
                            ctx_sb = ctx_pool.tile([HD + 1, 512], BF16,
                                                   tag="cs", name="cs")
                            nc.vector.tensor_copy(ctx_sb[:], cpss[qc][:])
                            # same-size buffer as cps: rotate in psC so the
                            # den matmul never blocks psA's projection chains
                            dn = psC.tile([HD + 1, 512], F32, tag="cps",
                                          name="dn")
                            nc.tensor.matmul(
                                dn[0:HD, :], selD[:, :], ctx_sb[:],
                                start=True, stop=True)
                            rc = rcp_pool.tile([HD, 512], F32, tag="rc",
                                               name="rc")
                            nc.vector.reciprocal(rc[:], dn[0:HD, :])
                            nc.vector.tensor_tensor(
                                out=dest[r0:r0 + HD,
                                         qc * 512:(qc + 1) * 512],
                                in0=ctx_sb[0:HD, :],
                                in1=rc[:], op=OP.mult)

                # ---- ship peer's token half to the pair core ------------
                nc.gpsimd.dma_start(agin[g][:, :], send[:])
                nc.gpsimd.collective_compute(
                    "AllGather", OP.bypass, replica_groups=GROUPS,
                    ins=[agin[g][:, :].opt()],
                    outs=[agout[g][:, :, :].opt()])

            # Fence: a 5th tiny AllGather.  The tile framework makes each
            # collective's input writer wait until all PRIOR collectives'
            # data has ARRIVED (the trigger instruction itself completes
            # early), so the fence writer + gpsimd program order make the
            # agout readbacks below race-free.
            fence_sb = const.tile([P, 8], BF16, tag="fsb", name="fsb")
            nc.vector.memset(fence_sb[:], 0.0)
            fence_in = dram.tile([P, 8], BF16, tag="fin", name="fin")
            fence_out = dram.tile([2, P, 8], BF16, tag="fout", name="fout")
            nc.gpsimd.dma_start(fence_in[:, :], fence_sb[:])
            nc.gpsimd.collective_compute(
                "AllGather", OP.bypass, replica_groups=GROUPS,
                ins=[fence_in[:, :].opt()],
                outs=[fence_out[:, :, :].opt()])

        # =============== Phase W: Wo + residual + LN1 ===================
        with tc.tile_pool(name="asb", bufs=1) as asb_pool, \
             tc.tile_pool(name="zT", bufs=1) as zT_pool, \
             tc.tile_pool(name="ln1", bufs=2) as ln1_pool, \
             tc.tile_pool(name="psW", bufs=3, space="PSUM") as psW, \
             tc.tile_pool(name="psStat", bufs=1, space="PSUM") as psStat, \
             tc.tile_pool(name="psBc", bufs=1, space="PSUM") as psBc:
            wo_sb = [wx_pool.tile([P, D], BF16, tag="wx", name="wx")
                     for _ in range(12)]
            for i in range(12):
                nc.sync.dma_start(out=wo_sb[i], in_=wo[i * P:(i + 1) * P, :])
            # FFN hb=0 W1 preload (overlaps W phase)
            w1b0 = [w1p_pool.tile([P, D], BF16, tag="w1p", name="w1p")
                    for _ in range(DT)]
            for i in range(DT):
                nc.sync.dma_start(out=w1b0[i], in_=w1[i * P:(i + 1) * P,
                                                      0:1024])

            # peer ctx contributions from the AllGathers (both slots; the
            # useless slot's wo rows are zero)
            asb = []
            for g in range(HG):
                for s in range(2):
                    a = asb_pool.tile([P, T], BF16, tag=f"a{g}{s}",
                                      name=f"a{g}{s}")
                    # gpsimd queue: executes after the fence writer above,
                    # i.e. after every AllGather's data has arrived
                    nc.gpsimd.dma_start(a[:], agout[g][s, :, :])
                    asb.append(a)
            # moving operands for Wo in wo_sb row order:
            #   rows 0:512   -> own ctxn groups 0..3 (own tokens = cols 0:T)
            #   rows 512:1536 -> agout g0 s0, g0 s1, g1 s0, ... (g3 last so
            #   the last AllGather's wait overlaps the first 10 matmuls)
            movs = [ctxn[g][:] for g in range(HG)] + asb

            zT = [zT_pool.tile([P, T], BF16, tag=f"zT{i}", name=f"zT{i}")
                  for i in range(DT)]
            for ch in range(T // 512):
                for dout in range(DT):
                    ps = psW.tile([P, 512], F32, tag="psW", name="psW")
                    for din in range(12):
                        nc.tensor.matmul(
                            ps[:], wo_sb[din][:, dout * P:(dout + 1) * P],
                            movs[din][:, ch * 512:(ch + 1) * 512],
                            start=(din == 0), stop=(din == 11))
                    # z = attn_out + bo + x_resid
                    nc.vector.scalar_tensor_tensor(
                        zT[dout][:, ch * 512:(ch + 1) * 512], ps[:],
                        biases["bo_p"][:, dout:dout + 1],
                        xsb[dout][:, ch * 512:(ch + 1) * 512],
                        op0=OP.add, op1=OP.add)

            # ---- LN1 (feature-major; stats over partitions via PE) -----
            for ch in range(T // 512):
                sl = slice(ch * 512, (ch + 1) * 512)
                sum_ps = psStat.tile([1, 512], F32, tag="s", name="s")
                sq_ps = psStat.tile([1, 512], F32, tag="q", name="q")
                for dt_ in range(DT):
                    zsq = ln1_pool.tile([P, 512], BF16, tag="zsq",
                                        name="zsq")
                    nc.scalar.activation(zsq[:], zT[dt_][:, sl], AF.Square)
                    nc.tensor.matmul(sum_ps[:], ones_col[:], zT[dt_][:, sl],
                                     start=(dt_ == 0), stop=(dt_ == DT - 1))
                    nc.tensor.matmul(sq_ps[:], ones_col[:], zsq[:],
                                     start=(dt_ == 0), stop=(dt_ == DT - 1))
                mean = ln1_pool.tile([1, 512], F32, tag="mean", name="mean")
                nc.scalar.mul(mean[:], sum_ps[:], 1.0 / D)
                msq = ln1_pool.tile([1, 512], F32, tag="msq", name="msq")
                nc.scalar.mul(msq[:], sq_ps[:], 1.0 / D)
                m2 = ln1_pool.tile([1, 512], F32, tag="m2", name="m2")
                nc.vector.tensor_mul(m2[:], mean[:], mean[:])
                var = ln1_pool.tile([1, 512], F32, tag="var", name="var")
                nc.vector.tensor_sub(var[:], msq[:], m2[:])
                std = ln1_pool.tile([1, 512], F32, tag="std", name="std")
                nc.scalar.activation(std[:], var[:], AF.Sqrt,
                                     bias=eps_sb[0:1, :])
                rstd = ln1_pool.tile([1, 512], F32, tag="rstd", name="rstd")
                nc.vector.reciprocal(rstd[:], std[:])
                mean_r = ln1_pool.tile([1, 512], BF16, tag="meanr",
                                       name="meanr")
                nc.vector.tensor_copy(mean_r[:], mean[:])
                rstd_r = ln1_pool.tile([1, 512], BF16, tag="rstdr",
                                       name="rstdr")
                nc.vector.tensor_copy(rstd_r[:], rstd[:])
                bm_ps = psBc.tile([P, 512], F32, tag="bm", name="bm")
                nc.tensor.matmul(bm_ps[:], ones_row[:], mean_r[:],
                                 start=True, stop=True)
                br_ps = psBc.tile([P, 512], F32, tag="br", name="br")
                nc.tensor.matmul(br_ps[:], ones_row[:], rstd_r[:],
                                 start=True, stop=True)
                bm = ln1_pool.tile([P, 512], F32, tag="bm_sb", name="bm_sb")
                nc.scalar.copy(bm[:], bm_ps[:])
                br = ln1_pool.tile([P, 512], F32, tag="br_sb", name="br_sb")
                nc.scalar.copy(br[:], br_ps[:])
                for dt_ in range(DT):
                    tmp = ln1_pool.tile([P, 512], F32, tag="n1", name="n1")
                    nc.vector.scalar_tensor_tensor(
                        tmp[:], zT[dt_][:, sl],
                        1.0, bm[:], op0=OP.mult, op1=OP.subtract)
                    tmp2 = ln1_pool.tile([P, 512], F32, tag="n2", name="n2")
                    nc.vector.scalar_tensor_tensor(
                        tmp2[:], tmp[:],
                        biases["g1_p"][:, dt_:dt_ + 1], br[:],
                        op0=OP.mult, op1=OP.mult)
                    nc.scalar.activation(
                        hT[dt_][:, sl], tmp2[:], AF.Identity,
                        bias=biases["be1_p"][:, dt_:dt_ + 1])

        es_fw.close()   # free x / QKV weights / ctx SBUF before the FFN

        # =============== Phase 4: FFN + residual + fused LN2/out ========
        with tc.tile_pool(name="wxf", bufs=22) as wxf_pool, \
             tc.tile_pool(name="z2T", bufs=1) as z2T_pool, \
             tc.tile_pool(name="t1", bufs=12) as t1_pool, \
             tc.tile_pool(name="ztmp", bufs=2) as ztmp_pool, \
             tc.tile_pool(name="o2", bufs=1) as o2_pool, \
             tc.tile_pool(name="tm", bufs=2) as tm_pool, \
             tc.tile_pool(name="lnc", bufs=1) as lnc_pool, \
             tc.tile_pool(name="ln2", bufs=2) as ln2_pool, \
             tc.tile_pool(name="psF1", bufs=3, space="PSUM") as psF1, \
             tc.tile_pool(name="psF2", bufs=3, space="PSUM") as psF2, \
             tc.tile_pool(name="psT5", bufs=2, space="PSUM") as psT5:
            z2T = [z2T_pool.tile([P, T], BF16, tag=f"z2T{i}", name=f"z2T{i}")
                   for i in range(DT)]
            out2 = [o2_pool.tile([P, T], F32, tag=f"o2{i}", name=f"o2{i}")
                    for i in range(DT)]
            g2_bc = lnc_pool.tile([P, D], F32, tag="g2bc", name="g2bc")
            nc.sync.dma_start(out=g2_bc, in_=bass.AP(
                tensor=t["g2_d"], offset=0, ap=[[0, P], [1, D]]))
            be2_bc = lnc_pool.tile([P, D], F32, tag="be2bc", name="be2bc")
            nc.sync.dma_start(out=be2_bc, in_=bass.AP(
                tensor=t["be2_d"], offset=0, ap=[[0, P], [1, D]]))

            def phase5(nts):
                for nt in nts:
                    z2 = tm_pool.tile([P, D], F32, tag="z2tm", name="z2tm")
                    for dt_ in range(DT):
                        pt = psT5.tile([P, P], BF16, tag="psT5", name="psT5")
                        nc.tensor.transpose(
                            pt[:], z2T[dt_][:, nt * P:(nt + 1) * P],
                            ident[:])
                        nc.scalar.copy(z2[:, dt_ * P:(dt_ + 1) * P], pt[:])
                    stats = ln2_pool.tile([P, 2, 6], F32, tag="st", name="st")
                    for gg in range(2):
                        nc.vector.bn_stats(out=stats[:, gg, :],
                                           in_=z2[:, gg * 512:(gg + 1) * 512])
                    mv = ln2_pool.tile([P, 2], F32, tag="mv", name="mv")
                    nc.vector.bn_aggr(out=mv[:], in_=stats[:])
                    std = ln2_pool.tile([P, 1], F32, tag="std2", name="std2")
                    nc.scalar.activation(std[:], mv[:, 1:2], AF.Sqrt,
                                         bias=eps_sb[:])
                    rstd = ln2_pool.tile([P, 1], F32, tag="rstd2",
                                         name="rstd2")
                    nc.vector.reciprocal(rstd[:], std[:])
                    xn = ln2_pool.tile([P, D], F32, tag="xn", name="xn")
                    nc.vector.tensor_scalar(
                        out=xn[:], in0=z2[:], scalar1=mv[:, 0:1],
                        scalar2=rstd[:], op0=OP.subtract, op1=OP.mult)
                    xg = ln2_pool.tile([P, D], F32, tag="xg", name="xg")
                    nc.vector.tensor_mul(xg[:], xn[:], g2_bc[:])
                    fin = ln2_pool.tile([P, D], F32, tag="fin", name="fin")
                    nc.vector.tensor_add(fin[:], xg[:], be2_bc[:])
                    nc.sync.dma_start(out=out[nt * P:(nt + 1) * P, :],
                                      in_=fin[:])

            for hb in range(4):              # hidden blocks of 1024
                if hb == 0:
                    w1b = w1b0
                else:
                    w1b = [wxf_pool.tile([P, D], BF16, tag="wxf", name="wxf")
                           for _ in range(DT)]
                    for i in range(DT):
                        nc.sync.dma_start(
                            out=w1b[i],
                            in_=w1[i * P:(i + 1) * P,
                                   hb * 1024:(hb + 1) * 1024])
                w2b = [wxf_pool.tile([P, D], BF16, tag="wxf", name="wxf")
                       for _ in range(DT)]
                for i in range(DT):
                    nc.sync.dma_start(
                        out=w2b[i],
                        in_=w2[(hb * 8 + i) * P:(hb * 8 + i + 1) * P, :])
                for tc4 in range(T // 512):  # 2 token chunks of 512
                    tsl = slice(tc4 * 512, (tc4 + 1) * 512)
                    t1s = []
                    for i in range(DT):      # 8 hidden tiles in block
                        t1ps = psF1.tile([P, 512], F32, tag="t1ps",
                                         name="t1ps")
                        for din in range(DT):
                            nc.tensor.matmul(
                                t1ps[:], w1b[din][:, i * P:(i + 1) * P],
                                hT[din][:, tsl],
                                start=(din == 0), stop=(din == DT - 1))
                        t1 = t1_pool.tile([P, 512], BF16, tag="t1", name="t1")
                        nc.scalar.activation(
                            t1[:], t1ps[:], AF.Relu,
                            bias=b1_sb[:, hb * 8 + i:hb * 8 + i + 1])
                        t1s.append(t1)
                    for dout in range(DT):
                        o2ps = psF2.tile([P, 512], F32, tag="o2ps",
                                         name="o2ps")
                        for i in range(DT):
                            nc.tensor.matmul(
                                o2ps[:], w2b[i][:, dout * P:(dout + 1) * P],
                                t1s[i][:],
                                start=(i == 0), stop=(i == DT - 1))
                        if hb == 0:
                            nc.vector.tensor_copy(out2[dout][:, tsl],
                                                  o2ps[:])
                        elif hb < 3:
                            nc.vector.tensor_tensor(
                                out=out2[dout][:, tsl], in0=o2ps[:],
                                in1=out2[dout][:, tsl], op=OP.add)
                        else:
                            # last block: fold in residual + b2 -> z2T
                            tmp = ztmp_pool.tile([P, 512], F32, tag="zt",
                                                 name="zt")
                            nc.vector.tensor_tensor(
                                out=tmp[:], in0=o2ps[:],
                                in1=out2[dout][:, tsl], op=OP.add)
                            nc.vector.scalar_tensor_tensor(
                                z2T[dout][:, tsl], tmp[:],
                                biases["b2_p"][:, dout:dout + 1],
                                hT[dout][:, tsl], op0=OP.add, op1=OP.add)
                    if hb == 3:
                        # LN2 + output for this half while the other half
                        # of the FFN (or nothing) still runs
                        phase5(range(tc4 * 4, (tc4 + 1) * 4))


def _selD():
    sel = np.zeros((HD + 1, HD), dtype=np.float32)
    sel[HD, :] = 1.0
    return sel


def _pack(v, nt):
    return np.ascontiguousarray(v.reshape(nt, P).T)


def kernel(x, Wq, bq, Wk, bk, Wv, bv, Wo, bo, W1, b1, W2, b2, g1, beta1,
           g2, beta2):
    x = np.asarray(x, dtype=np.float32)
    if "nc" not in _CACHED:
        _CACHED["nc"] = _build_program()
    nc = _CACHED["nc"]

    import ml_dtypes
    bf16 = lambda a: np.ascontiguousarray(
        np.asarray(a, dtype=np.float32).astype(ml_dtypes.bfloat16))
    f32 = lambda a: np.ascontiguousarray(np.asarray(a, dtype=np.float32))
    scale = 1.0 / np.sqrt(HD)
    Wk_s = np.asarray(Wk, np.float64) * scale
    bk_s = f32(bk) * scale

    common = {
        "w1": bf16(W1), "w2": bf16(W2),
        "bo_p": _pack(f32(bo), DT), "b1_p": _pack(f32(b1), FT),
        "b2_p": _pack(f32(b2), DT),
        "g1_p": _pack(f32(g1), DT), "be1_p": _pack(f32(beta1), DT),
        "g2_d": f32(g2), "be2_d": f32(beta2),
        "ident_d": np.eye(P).astype(ml_dtypes.bfloat16),
        "ones_row_d": np.ones((1, P)).astype(ml_dtypes.bfloat16),
        "ones_col_d": np.ones((P, 1)).astype(ml_dtypes.bfloat16),
        "selD_d": _selD().astype(ml_dtypes.bfloat16),
    }
    in_maps = []
    for c in range(N_CORES):
        b, half = c // 2, c % 2
        own = x[b, half * T:(half + 1) * T]           # [1024, 1024]
        other = x[b, (1 - half) * T:(2 - half) * T]
        xT_c = np.ascontiguousarray(
            np.concatenate([own, other], axis=0).T).astype(
                ml_dtypes.bfloat16)                   # [1024, 2048]
        hsl = slice(half * 512, (half + 1) * 512)
        # [D, 512] -> [D, 4, 128] (group-major, natural order)
        wq_c = bf16(np.asarray(Wq)[:, hsl]).reshape(D, HG, P)
        wk_c = bf16(Wk_s[:, hsl]).reshape(D, HG, P)
        wv_c = bf16(np.asarray(Wv)[:, hsl]).reshape(D, HG, P)
        # wo rows: own 512 (natural), then per group g: slot0, slot1 where
        # slot s holds Wo rows of heads (s*8 + 2g, s*8+2g+1) if s != half
        # else zeros (that slot of the AllGather is this core's own data
        # for the peer's tokens -- not used here).
        Wo_np = np.asarray(Wo, np.float32)
        rows = [Wo_np[hsl]]
        for g in range(HG):
            for s in range(2):
                if s != half:
                    rows.append(Wo_np[s * 512 + g * P: s * 512 + (g + 1) * P])
                else:
                    rows.append(np.zeros((P, D), np.float32))
        wo_c = bf16(np.concatenate(rows, axis=0))     # [1536, 1024]
        bqp = _pack(f32(bq)[hsl], HG)
        bkp = _pack(bk_s[hsl], HG)
        bvp = _pack(f32(bv)[hsl], HG)
        in_maps.append({**common, "xT": np.ascontiguousarray(xT_c),
                        "wq": np.ascontiguousarray(wq_c),
                        "wk": np.ascontiguousarray(wk_c),
                        "wv": np.ascontiguousarray(wv_c),
                        "wo": wo_c,
                        "bq_p": bqp, "bk_p": bkp, "bv_p": bvp})

    trace = bool(os.environ.get("KERNEL_TRACE"))
    res = bass_utils.run_bass_kernel_spmd(
        nc, in_maps, core_ids=list(range(N_CORES)), trace=trace)
    _CACHED["last_result"] = res

    y = np.empty((4, S, D), dtype=np.float32)
    for c in range(N_CORES):
        b, half = c // 2, c % 2
        y[b, half * T:(half + 1) * T] = res.results[c]["out"]
    return y
